# revision 56
# baseline (speedup 1.0000x reference)
"""Trainium2 Bass kernel for BA3MotifNet (4-layer LEConv GNN + mean-pool + MLP).

SPMD across 8 NeuronCores, single compiled graph; all per-core variation is
carried in the input data (index streams), never in instruction structure.

  - Nodes dst-sharded at graph boundaries (batch sorted): core c owns graphs
    [125c,125(c+1)) and their nodes, padded to NODE_PAD=12800/core.
  - Per layer: a = h@W1+b1 computed node-major (nodes in PSUM partition dim),
    a ones-column rides element 64 of each 256B bf16 row; DMA'd to DRAM,
    AllGather -> a_full [102400,128] (Shared, one buffer per layer).
  - agg_i = sum_{e:dst=i} ew_e*a[src_e] - (h@W2)_i * degw_i.
    Tile stream is WINDOW-MAJOR: for each 128-dst window, the 4 source
    quarters' tiles run consecutively, accumulating in one PSUM region;
    a single scalar-engine Copy flushes agg (+deg row on layer 0) to aggT.
    Gather: one SWDGE dma_gather per (window, quarter) run of 256B rows from
    a_full, round-robin across 4 SWDGE queues; int16 indices.
    Scatter: PE matmul aggT[f,d] += gathered[e,f].T @ onehot[e,d]; onehot
    [128, LS_T*128] groups built on DVE by broadcast tensor_tensor
    (iota==sidx)*ew -- no PSUM-coupled DVE ops anywhere in the stream.
  - h = relu(aggT - t2T*dgw) on 512-col slabs; layer 3 produces node-major
    bf16 h staged via DRAM for pooling.
  - Mean-pool via (1/cnt)-valued one-hot matmuls; 2-layer MLP on-core.
  - Out: per-core [128,4] f32 -> host concat -> [1000,3].
"""

import os
import sys

import numpy as np

sys.path.insert(0, "/opt/trn_rl_repo")

ABL_GATHER = os.environ.get("ABL_GATHER", "0") == "1"   # memset instead of gather
ABL_LSCAT = os.environ.get("ABL_LSCAT", "0") == "1"     # memset instead of onehot
ABL_AG = os.environ.get("ABL_AG", "0") == "1"           # skip AllGather collective
ABL_MM = os.environ.get("ABL_MM", "0") == "1"           # skip per-tile matmuls
ABL_DENSE = os.environ.get("ABL_DENSE", "0") == "1"     # skip dense a/t2 phase
ABL_COMB = os.environ.get("ABL_COMB", "0") == "1"       # skip combine + dgw
ABL_TAIL = os.environ.get("ABL_TAIL", "0") == "1"       # skip pooling + MLP
ABL_FLUSH = os.environ.get("ABL_FLUSH", "0") == "1"     # skip W3b/flush/psC
ABL_GIX = os.environ.get("ABL_GIX", "0") == "1"         # skip gix loads

FULL_CFG = dict(
    n_nodes=100000, n_edges=3200000, n_graphs=1000, hid=64, n_layers=4,
    nc=8, node_pad=12800, ls_t=28,
)


# --------------------------------------------------------------- host prep
def shard_and_pack(inputs, cfg):
    NC, NP = cfg["nc"], cfg["node_pad"]
    NW, NQ = NP // 128, 4
    QROWS = NP * NC // NQ
    G = cfg["n_graphs"]
    GPC = G // NC
    assert QROWS <= 32768

    x = np.asarray(inputs["x"], np.float32)
    ei = np.asarray(inputs["edge_index"], np.int64)
    ew = np.asarray(inputs["edge_attr"], np.float32)
    batch = np.asarray(inputs["batch"], np.int64)
    N = x.shape[0]
    NF = x.shape[1]

    gs = np.searchsorted(batch, np.arange(G + 1))
    nstart = gs[np.arange(NC + 1) * GPC]
    ncnt = np.diff(nstart)
    if ncnt.max() > NP:                                    # rare: grow pad
        NP = int(-(-int(ncnt.max()) // 512) * 512)
        cfg = dict(cfg, node_pad=NP)
        NW = NP // 128
        QROWS = NP * NC // NQ
        assert QROWS <= 32768

    shard_of = np.searchsorted(nstart[1:], np.arange(N), side="right")
    src, dst = ei[0], ei[1]
    e_core = shard_of[dst]
    # quarter of a src node depends only on its shard (QROWS == 2*NP)
    e_q = shard_of[src] * NP // QROWS

    # degree-balanced window packing per core: relabel local node ids so every
    # (window, quarter) edge count is as even as possible.
    newloc = np.zeros(N, np.int64)
    for c in range(NC):
        n_c = int(ncnt[c])
        deg4 = np.zeros((NP, NQ), np.int64)
        selc = e_core == c
        np.add.at(deg4, (dst[selc] - nstart[c], e_q[selc]), 1)
        deg4 = deg4[:n_c]
        order = np.argsort(-deg4.sum(1), kind="stable")
        loads = np.zeros((NW, NQ), np.int64)
        fill = np.zeros(NW, np.int64)
        assign = np.zeros(n_c, np.int64)
        for n in order:
            new_loads = loads + deg4[n]
            over = np.maximum(0, new_loads - 1016).sum(1)
            cand = over * 1e6 + new_loads.max(1).astype(np.float64)
            cand[fill >= 128] = np.inf
            wsel = int(np.argmin(cand))
            assign[n] = wsel * 128 + fill[wsel]
            fill[wsel] += 1
            loads[wsel] += deg4[n]
        newloc[nstart[c]: nstart[c] + n_c] = assign

    spad = shard_of * NP + newloc
    dstloc = newloc[dst]
    e_w = dstloc >> 7

    cnt = np.zeros((NC, NW, NQ), np.int64)
    np.add.at(cnt, (e_core, e_w, e_q), 1)
    T = np.maximum(1, -(-cnt.max(axis=0) // 128))          # [NW, NQ]

    ntiles = int(T.sum())
    LS_T = cfg["ls_t"]
    ntiles_pad = -(-ntiles // LS_T) * LS_T

    per_core = []
    for c in range(NC):
        sel = e_core == c
        s_qi = (spad[src[sel]] % QROWS).astype(np.int64)
        s_q, s_w = e_q[sel], e_w[sel]
        s_off = (dstloc[sel] & 127).astype(np.int64)
        s_ew = ew[sel]

        # window-major slot blocks: tiles ordered (w, q)
        order = np.lexsort((s_off, s_q, s_w))
        s_qi, s_q, s_w, s_off, s_ew = (a[order] for a in (s_qi, s_q, s_w, s_off, s_ew))
        blk_sizes = (T.reshape(-1) * 128)
        blk_base = np.concatenate([[0], np.cumsum(blk_sizes)])[:-1].reshape(NW, NQ)
        key = s_w * NQ + s_q
        grp_start = np.searchsorted(key, np.arange(NW * NQ), side="left")
        slot = blk_base[s_w, s_q] + (np.arange(key.size) - grp_start[key])

        nslots = ntiles * 128
        gidx = np.zeros(nslots, np.int16)
        ewv = np.zeros(nslots, np.float32)
        offv = np.full(nslots, -1, np.int64)
        gidx[slot] = s_qi.astype(np.int16)
        ewv[slot] = s_ew
        offv[slot] = s_off

        gw = np.tile(gidx.reshape(-1, 16).T, (8, 1))       # [128, nslots/16]

        offm = offv.reshape(ntiles, 128).T
        ewm = ewv.reshape(ntiles, 128).T
        sidx = offm.astype(np.int16)          # dst offset in window, -1 = pad
        sidx = np.pad(sidx, ((0, 0), (0, ntiles_pad - ntiles)), constant_values=-1)
        ewm = np.pad(ewm, ((0, 0), (0, ntiles_pad - ntiles)))

        loc = newloc[nstart[c]: nstart[c + 1]]
        xT1 = np.zeros((NF + 1, NP), np.float32)
        xT1[:NF, loc] = x[nstart[c]: nstart[c + 1]].T
        xT1[NF, :] = 1.0

        nb = (batch[nstart[c]: nstart[c + 1]] - c * GPC).astype(np.int64)
        cnts = np.bincount(nb, minlength=GPC).astype(np.float32)
        pool = np.zeros((128, NP), np.float32)
        pool[loc & 127, (loc >> 7) * 128 + nb] = 1.0 / np.maximum(cnts[nb], 1.0)

        per_core.append(dict(gidx=gw, sidx=sidx, ew=ewm, xT1=xT1, pool=pool,
                             ngraphs=GPC))

    meta = dict(T=T, ntiles=ntiles, ntiles_pad=ntiles_pad,
                NW=NW, NQ=NQ, QROWS=QROWS, NF=NF, cfg=cfg)
    return per_core, meta


def weights_map(inputs):
    f32 = np.float32
    vs = np.vstack
    w = {"embWb": vs([np.asarray(inputs["emb_w"], f32),
                      np.asarray(inputs["emb_b"], f32)[None]]),
         "L1b": vs([np.asarray(inputs["lin1_w"], f32),
                    np.asarray(inputs["lin1_b"], f32)[None]]),
         "L2b": vs([np.asarray(inputs["lin2_w"], f32),
                    np.asarray(inputs["lin2_b"], f32)[None]]),
         "ident": np.eye(128, dtype=f32),
         "iotaG": np.tile(np.tile(np.arange(128, dtype=f32),
                          FULL_CFG["ls_t"]), (128, 1))}
    L = np.asarray(inputs["conv_w1"]).shape[0]
    for l in range(L):
        w[f"W1b_{l}"] = vs([np.asarray(inputs["conv_w1"][l], f32),
                            np.asarray(inputs["conv_b1"][l], f32)[None]])
        w[f"W2_{l}"] = np.asarray(inputs["conv_w2"][l], f32)
        w3 = vs([np.asarray(inputs["conv_w3"][l], f32),
                 np.asarray(inputs["conv_b3"][l], f32)[None]])
        w[f"W3b_{l}"] = np.hstack([w3, np.zeros((w3.shape[0], 1), f32)]) \
            if l == 0 else w3
    return w


# --------------------------------------------------------------- builder
def build_graph(meta):
    from concourse import bacc, mybir, tile

    cfg = meta["cfg"]
    NC, H, L = cfg["nc"], cfg["hid"], cfg["n_layers"]
    NP, NW, NQ, QROWS = cfg["node_pad"], meta["NW"], meta["NQ"], meta["QROWS"]
    NF = meta["NF"]
    H2 = 2 * H                                  # padded bf16 a-row (256B)
    T = meta["T"]
    ntiles, ntiles_pad = meta["ntiles"], meta["ntiles_pad"]
    LS_T = cfg["ls_t"]
    TMAXQ = int(T.max())
    TWMAX = int(T.sum(axis=1).max())
    f32, bf16, i16 = mybir.dt.float32, mybir.dt.bfloat16, mybir.dt.int16
    AF = mybir.ActivationFunctionType
    NCH = NP // 512

    nc = bacc.Bacc(num_devices=NC, num_swdge_queues=4)

    gidx_d = nc.declare_dram_parameter("gidx", [128, ntiles * 8], i16, False)
    sidx_d = nc.declare_dram_parameter("sidx", [128, ntiles_pad], bf16, False)
    ew_d = nc.declare_dram_parameter("ew", [128, ntiles_pad], bf16, False)
    xT1_d = nc.declare_dram_parameter("xT1", [NF + 1, NP], bf16, False)
    pool_d = nc.declare_dram_parameter("pool", [128, NP], bf16, False)
    wnames = (["embWb", "L1b", "L2b", "ident", "iotaG"]
              + [f"{p}_{l}" for l in range(L) for p in ("W1b", "W2", "W3b")])
    wshape = {"embWb": [NF + 1, H], "L1b": [H + 1, H], "L2b": [H + 1, 3],
              "ident": [128, 128], "iotaG": [128, LS_T * 128]}
    wdt = {"embWb": bf16, "L1b": f32, "L2b": f32, "ident": f32, "iotaG": bf16}
    for l in range(L):
        wshape[f"W1b_{l}"] = [H + 1, H]
        wshape[f"W2_{l}"] = [H, H]
        wshape[f"W3b_{l}"] = [H + 1, H + 1] if l == 0 else [H + 1, H]
        wdt[f"W1b_{l}"] = wdt[f"W2_{l}"] = wdt[f"W3b_{l}"] = bf16
    wd = {k: nc.declare_dram_parameter(k, wshape[k], wdt[k], False)
          for k in wnames}
    out_d = nc.declare_dram_parameter("out", [128, 4], f32, True)

    with tile.TileContext(nc) as tc:
        with (
            tc.tile_pool(name="res", bufs=1) as res,
            tc.tile_pool(name="dram", bufs=1, space="DRAM") as dram,
            tc.tile_pool(name="stage", bufs=2) as stage,
            tc.tile_pool(name="gbuf", bufs=10) as gpool,
            tc.tile_pool(name="ohbuf", bufs=6) as ohpool,
            tc.tile_pool(name="ixbuf", bufs=3) as ixpool,
            tc.tile_pool(name="scr", bufs=2) as scr,
            tc.tile_pool(name="psA", bufs=2, space="PSUM") as psA,
            tc.tile_pool(name="psB", bufs=2, space="PSUM") as psB,
            tc.tile_pool(name="psC", bufs=3, space="PSUM") as psC,
            tc.tile_pool(name="psG", bufs=1, space="PSUM") as psG,
        ):
            a_loc_pp = [dram.tile([NP, H2], bf16, name=f"a_loc{i}")
                        for i in range(2)]
            a_full_pp = [dram.tile([NP * NC, H2], bf16, name=f"a_full{i}",
                                   addr_space="Shared")
                         for i in range(4)]
            h_nm_d = dram.tile([NW, 128, H], bf16)

            hT = res.tile([H + 1, NP], bf16, tag="hT")
            t2T = res.tile([H, NP], bf16, tag="t2T")
            aggT = res.tile([H + 1, NP], bf16, tag="aggT")
            dgw = res.tile([H, NP], bf16, tag="dgw")
            sidx_s = res.tile([128, ntiles_pad], bf16, tag="sidx")
            ew_s = res.tile([128, ntiles_pad], bf16, tag="ew")
            ones_row = res.tile([1, H], bf16, tag="ones_row")
            wts = {k: res.tile(wshape[k], wdt[k], tag=k, name=k) for k in wnames}
            ident = wts["ident"]

            nc.vector.memset(ones_row[:], 1.0)
            for k in wnames:
                nc.sync.dma_start(wts[k][:], wd[k][:])
            nc.sync.dma_start(sidx_s[:], sidx_d[:])
            nc.sync.dma_start(ew_s[:], ew_d[:])

            # h0T = embWb.T @ xT1 (streamed)
            for ck in range(NCH):
                sl = slice(ck * 512, (ck + 1) * 512)
                xc = stage.tile([NF + 1, 512], bf16, tag="xc")
                nc.sync.dma_start(xc[:], xT1_d[:, sl])
                ps = psA.tile([H, 512], f32, tag="dps")
                nc.tensor.matmul(ps[:], wts["embWb"][:], xc[:],
                                 start=True, stop=True)
                nc.scalar.activation(hT[:H, sl], ps[:], AF.Copy)
            nc.vector.memset(hT[H:H + 1, :], 1.0)

            for l in range(L):
                layer0 = l == 0
                HD = H + 1 if layer0 else H
                a_loc = a_loc_pp[l % 2]
                a_full = a_full_pp[l]

                # ---- dense: a (node-major, ones col at elem 64) -> a_loc; t2T
                for ck in range(NCH if not ABL_DENSE else 0):
                    sl = slice(ck * 512, (ck + 1) * 512)
                    ast = stage.tile([128, 4, H2], bf16, tag="ast")
                    nc.vector.memset(ast[:, :, H:], 0.0)
                    nc.vector.memset(ast[:, :, H:H + 1], 1.0)
                    for j in range(4):
                        wsl = slice(ck * 512 + j * 128, ck * 512 + (j + 1) * 128)
                        pst = psB.tile([128, H], f32, tag="tps")
                        nc.tensor.matmul(pst[:], hT[:, wsl], wts[f"W1b_{l}"][:],
                                         start=True, stop=True)
                        nc.vector.tensor_copy(ast[:, j, :H], pst[:])
                    nc.sync.dma_start(
                        a_loc[sl, :].rearrange("(j p) f -> p j f", p=128), ast[:])
                    ps2 = psA.tile([H, 512], f32, tag="dps")
                    nc.tensor.matmul(ps2[:], wts[f"W2_{l}"][:], hT[:H, sl],
                                     start=True, stop=True)
                    nc.scalar.activation(t2T[:, sl], ps2[:], AF.Copy)

                if not ABL_AG:
                    nc.gpsimd.collective_compute(
                        "AllGather", mybir.AluOpType.bypass,
                        replica_groups=[list(range(NC))],
                        ins=[a_loc[:].opt()], outs=[a_full[:].opt()])

                # ---- window-major gather+scatter stream
                t = 0
                gcall = 0
                ohb = None
                for w in range(NW):
                    wsl = slice(w * 128, (w + 1) * 128)
                    Tw = int(T[w].sum())
                    if not ABL_FLUSH:
                        ps = psC.tile([H + 1, 128], f32, tag="sps")
                        nc.tensor.matmul(ps[:HD, :], wts[f"W3b_{l}"][:],
                                         hT[:, wsl], start=True, stop=False)
                    if not ABL_GIX:
                        gix = ixpool.tile([128, TWMAX * 8], i16, tag="gix")
                        nc.sync.dma_start(gix[:, :Tw * 8],
                                          gidx_d[:, t * 8:(t + Tw) * 8])
                    toff = 0
                    for q in range(NQ):
                        Twq = int(T[w][q])
                        gb = gpool.tile([128, TMAXQ, H2], bf16, tag="gb")
                        if ABL_GATHER:
                            nc.vector.memset(gb[:, :Twq, :], 0.01)
                        else:
                            nc.gpsimd.dma_gather(
                                gb[:, :Twq, :],
                                a_full[q * QROWS:(q + 1) * QROWS, :],
                                gix[:, toff * 8:(toff + Twq) * 8],
                                Twq * 128, Twq * 128, H2,
                                single_packet=False, queue_num=gcall % 4)
                            gcall += 1
                        for i in range(Twq):
                            if t % LS_T == 0:
                                ohb = ohpool.tile([128, LS_T * 128], bf16,
                                                  tag="ohb")
                                g0 = t
                                if ABL_LSCAT:
                                    nc.vector.memset(ohb[:], 0.001)
                                else:
                                    oh3 = ohb[:].rearrange(
                                        "p (t c) -> p t c", c=128)
                                    sib = (sidx_s[:, g0:g0 + LS_T]
                                           .rearrange("p (t o) -> p t o", o=1)
                                           .broadcast_to([128, LS_T, 128]))
                                    ewb = (ew_s[:, g0:g0 + LS_T]
                                           .rearrange("p (t o) -> p t o", o=1)
                                           .broadcast_to([128, LS_T, 128]))
                                    it3 = wts["iotaG"][:].rearrange(
                                        "p (t c) -> p t c", c=128)
                                    nc.vector.tensor_tensor(
                                        oh3, it3, sib, mybir.AluOpType.is_equal)
                                    nc.vector.tensor_tensor(
                                        oh3, oh3, ewb, mybir.AluOpType.mult)
                            oh_sl = ohb[:, (t % LS_T) * 128:(t % LS_T + 1) * 128]
                            last = (q == NQ - 1) and (i == Twq - 1)
                            if not ABL_FLUSH and (not ABL_MM or last):
                                nc.tensor.matmul(
                                    ps[:HD, :], gb[:, i, :HD], oh_sl,
                                    start=False, stop=last)
                            t += 1
                            toff += 1
                    if not ABL_FLUSH:
                        nc.scalar.activation(aggT[:HD, wsl], ps[:HD, :], AF.Copy)

                if layer0 and not ABL_COMB:
                    # dgw = PE-broadcast of deg row (via partition-0 staging)
                    for ck in range(NCH):
                        sl = slice(ck * 512, (ck + 1) * 512)
                        dr = stage.tile([1, 512], bf16, tag="dr")
                        nc.sync.dma_start(dr[:], aggT[H:H + 1, sl])
                        psr = psA.tile([H, 512], f32, tag="dps")
                        nc.tensor.matmul(psr[:], ones_row[:], dr[:],
                                         start=True, stop=True)
                        nc.scalar.activation(dgw[:, sl], psr[:], AF.Copy)

                # ---- combine: h = relu(aggT - t2T*dgw)
                if ABL_COMB:
                    continue
                if l < L - 1:
                    for ck in range(NCH):
                        csl = slice(ck * 512, (ck + 1) * 512)
                        tmp = scr.tile([H, 512], f32, tag="cmb1")
                        nc.gpsimd.tensor_mul(tmp[:], t2T[:, csl], dgw[:, csl])
                        nc.gpsimd.tensor_sub(tmp[:], aggT[:H, csl], tmp[:])
                        nc.scalar.activation(hT[:H, csl], tmp[:], AF.Relu)
                else:
                    for w in range(NW):
                        wsl = slice(w * 128, (w + 1) * 128)
                        tmp = scr.tile([H, 128], f32, tag="cmb2")
                        nc.vector.tensor_mul(tmp[:], t2T[:, wsl], dgw[:, wsl])
                        nc.vector.tensor_sub(tmp[:], aggT[:H, wsl], tmp[:])
                        hTw = scr.tile([H, 128], f32, tag="cmb3")
                        nc.scalar.activation(hTw[:], tmp[:], AF.Relu)
                        pst = psB.tile([128, H], f32, tag="tps")
                        nc.tensor.transpose(pst[:], hTw[:], ident[:H, :H])
                        hst = scr.tile([128, H], bf16, tag="hst")
                        nc.vector.tensor_copy(hst[:], pst[:])
                        nc.sync.dma_start(h_nm_d[w], hst[:])

            # ---- pooling + MLP
            if ABL_TAIL:
                outs0 = stage.tile([128, 4], f32, tag="outs")
                nc.vector.memset(outs0[:], 0.0)
                nc.sync.dma_start(out_d[:], outs0[:])
            else:
                psg = psG.tile([128, H], f32, tag="spsg")
                for w in range(NW):
                    pw = ixpool.tile([128, 128], bf16, tag="pw")
                    nc.sync.dma_start(pw[:], pool_d[:, w * 128:(w + 1) * 128])
                    hb = ixpool.tile([128, H], bf16, tag="hb")
                    nc.sync.dma_start(hb[:], h_nm_d[w])
                    nc.tensor.matmul(psg[:], pw[:], hb[:],
                                     start=(w == 0), stop=(w == NW - 1))
                gx = stage.tile([128, H], f32, tag="gx")
                nc.vector.tensor_copy(gx[:], psg[:])
                pst = psB.tile([128, 128], f32, tag="tps")
                nc.tensor.transpose(pst[:H, :], gx[:], ident[:])
                gxT = stage.tile([H + 1, 128], f32, tag="gxT")
                nc.vector.tensor_copy(gxT[:H, :], pst[:H, :])
                nc.vector.memset(gxT[H:H + 1, :], 1.0)
                ps1 = psB.tile([128, H], f32, tag="tps")
                nc.tensor.matmul(ps1[:], gxT[:], wts["L1b"][:],
                                 start=True, stop=True)
                r1 = stage.tile([128, H], f32, tag="r1")
                nc.scalar.activation(r1[:], ps1[:], AF.Relu)
                pst2 = psB.tile([128, 128], f32, tag="tps")
                nc.tensor.transpose(pst2[:H, :], r1[:], ident[:])
                r1T = stage.tile([H + 1, 128], f32, tag="r1T")
                nc.vector.tensor_copy(r1T[:H, :], pst2[:H, :])
                nc.vector.memset(r1T[H:H + 1, :], 1.0)
                ps2 = psB.tile([128, 4], f32, tag="tps")
                nc.tensor.matmul(ps2[:, :3], r1T[:], wts["L2b"][:],
                                 start=True, stop=True)
                outs = stage.tile([128, 4], f32, tag="outs")
                nc.vector.memset(outs[:], 0.0)
                nc.vector.tensor_copy(outs[:, :3], ps2[:, :3])
                nc.sync.dma_start(out_d[:], outs[:])

    nc.compile()
    return nc


# --------------------------------------------------------------- entry
F32_KEYS = ("L1b", "L2b", "ident")


def make_in_maps(per_core, w, cfg):
    import ml_dtypes
    bf = ml_dtypes.bfloat16
    in_maps = []
    for c in range(cfg["nc"]):
        pc = per_core[c]
        m = {}
        for k, v in w.items():
            m[k] = v if k in F32_KEYS else v.astype(bf)
        m["gidx"], m["sidx"] = pc["gidx"], pc["sidx"].astype(bf)
        m["ew"] = pc["ew"].astype(bf)
        m["xT1"] = pc["xT1"].astype(bf)
        m["pool"] = pc["pool"].astype(bf)
        in_maps.append(m)
    return in_maps


def run(inputs, cfg, trace=False):
    per_core, meta = shard_and_pack(inputs, cfg)
    w = weights_map(inputs)
    in_maps = make_in_maps(per_core, w, cfg)
    nc = build_graph(meta)
    from concourse import bass_utils
    res = bass_utils.run_bass_kernel_spmd(
        nc, in_maps, core_ids=list(range(cfg["nc"])), trace=trace)
    outs = [np.asarray(res.results[c]["out"])[:per_core[c]["ngraphs"], :3]
            for c in range(cfg["nc"])]
    return np.concatenate(outs, 0).astype(np.float32), res


def kernel(**inputs):
    out, _ = run(inputs, FULL_CFG)
    return out


# revision 59
# speedup vs baseline: 1.0079x; 1.0079x over previous
"""Trainium2 Bass kernel for BA3MotifNet (4-layer LEConv GNN + mean-pool + MLP).

SPMD across 8 NeuronCores, single compiled graph; all per-core variation is
carried in the input data (index streams), never in instruction structure.

  - Nodes dst-sharded at graph boundaries (batch sorted): core c owns graphs
    [125c,125(c+1)) and their nodes, padded to NODE_PAD=12800/core.
  - Per layer: a = h@W1+b1 computed node-major (nodes in PSUM partition dim),
    a ones-column rides element 64 of each 256B bf16 row; DMA'd to DRAM,
    AllGather -> a_full [102400,128] (Shared, one buffer per layer).
  - agg_i = sum_{e:dst=i} ew_e*a[src_e] - (h@W2)_i * degw_i.
    Tile stream is WINDOW-MAJOR: for each 128-dst window, the 4 source
    quarters' tiles run consecutively, accumulating in one PSUM region;
    a single scalar-engine Copy flushes agg (+deg row on layer 0) to aggT.
    Gather: one SWDGE dma_gather per (window, quarter) run of 256B rows from
    a_full, round-robin across 4 SWDGE queues; int16 indices.
    Scatter: PE matmul aggT[f,d] += gathered[e,f].T @ onehot[e,d]; onehot
    [128, LS_T*128] groups built on DVE by broadcast tensor_tensor
    (iota==sidx)*ew -- no PSUM-coupled DVE ops anywhere in the stream.
  - h = relu(aggT - t2T*dgw) on 512-col slabs; layer 3 produces node-major
    bf16 h staged via DRAM for pooling.
  - Mean-pool via (1/cnt)-valued one-hot matmuls; 2-layer MLP on-core.
  - Out: per-core [128,4] f32 -> host concat -> [1000,3].
"""

import os
import sys

import numpy as np

sys.path.insert(0, "/opt/trn_rl_repo")

ABL_GATHER = os.environ.get("ABL_GATHER", "0") == "1"   # memset instead of gather
ABL_LSCAT = os.environ.get("ABL_LSCAT", "0") == "1"     # memset instead of onehot
ABL_AG = os.environ.get("ABL_AG", "0") == "1"           # skip AllGather collective
ABL_MM = os.environ.get("ABL_MM", "0") == "1"           # skip per-tile matmuls
ABL_DENSE = os.environ.get("ABL_DENSE", "0") == "1"     # skip dense a/t2 phase
ABL_COMB = os.environ.get("ABL_COMB", "0") == "1"       # skip combine + dgw
ABL_TAIL = os.environ.get("ABL_TAIL", "0") == "1"       # skip pooling + MLP
ABL_FLUSH = os.environ.get("ABL_FLUSH", "0") == "1"     # skip W3b/flush/psC
ABL_GIX = os.environ.get("ABL_GIX", "0") == "1"         # skip gix loads

FULL_CFG = dict(
    n_nodes=100000, n_edges=3200000, n_graphs=1000, hid=64, n_layers=4,
    nc=8, node_pad=12800, ls_t=28,
)


# --------------------------------------------------------------- host prep
def shard_and_pack(inputs, cfg):
    NC, NP = cfg["nc"], cfg["node_pad"]
    NW, NQ = NP // 128, 4
    QROWS = NP * NC // NQ
    G = cfg["n_graphs"]
    GPC = G // NC
    assert QROWS <= 32768

    x = np.asarray(inputs["x"], np.float32)
    ei = np.asarray(inputs["edge_index"], np.int64)
    ew = np.asarray(inputs["edge_attr"], np.float32)
    batch = np.asarray(inputs["batch"], np.int64)
    N = x.shape[0]
    NF = x.shape[1]

    gs = np.searchsorted(batch, np.arange(G + 1))
    nstart = gs[np.arange(NC + 1) * GPC]
    ncnt = np.diff(nstart)
    if ncnt.max() > NP:                                    # rare: grow pad
        NP = int(-(-int(ncnt.max()) // 512) * 512)
        cfg = dict(cfg, node_pad=NP)
        NW = NP // 128
        QROWS = NP * NC // NQ
        assert QROWS <= 32768

    shard_of = np.searchsorted(nstart[1:], np.arange(N), side="right")
    src, dst = ei[0], ei[1]
    e_core = shard_of[dst]
    # quarter of a src node depends only on its shard (QROWS == 2*NP)
    e_q = shard_of[src] * NP // QROWS

    # degree-balanced window packing per core: relabel local node ids so every
    # (window, quarter) edge count is as even as possible.
    newloc = np.zeros(N, np.int64)
    for c in range(NC):
        n_c = int(ncnt[c])
        deg4 = np.zeros((NP, NQ), np.int64)
        selc = e_core == c
        np.add.at(deg4, (dst[selc] - nstart[c], e_q[selc]), 1)
        deg4 = deg4[:n_c]
        order = np.argsort(-deg4.sum(1), kind="stable")
        loads = np.zeros((NW, NQ), np.int64)
        fill = np.zeros(NW, np.int64)
        assign = np.zeros(n_c, np.int64)
        for n in order:
            new_loads = loads + deg4[n]
            over = np.maximum(0, new_loads - 1016).sum(1)
            cand = over * 1e6 + new_loads.max(1).astype(np.float64)
            cand[fill >= 128] = np.inf
            wsel = int(np.argmin(cand))
            assign[n] = wsel * 128 + fill[wsel]
            fill[wsel] += 1
            loads[wsel] += deg4[n]
        newloc[nstart[c]: nstart[c] + n_c] = assign

    spad = shard_of * NP + newloc
    dstloc = newloc[dst]
    e_w = dstloc >> 7

    cnt = np.zeros((NC, NW, NQ), np.int64)
    np.add.at(cnt, (e_core, e_w, e_q), 1)
    T = np.maximum(1, -(-cnt.max(axis=0) // 128))          # [NW, NQ]

    ntiles = int(T.sum())
    LS_T = cfg["ls_t"]
    ntiles_pad = -(-ntiles // LS_T) * LS_T

    per_core = []
    for c in range(NC):
        sel = e_core == c
        s_qi = (spad[src[sel]] % QROWS).astype(np.int64)
        s_q, s_w = e_q[sel], e_w[sel]
        s_off = (dstloc[sel] & 127).astype(np.int64)
        s_ew = ew[sel]

        # window-major slot blocks: tiles ordered (w, q)
        order = np.lexsort((s_off, s_q, s_w))
        s_qi, s_q, s_w, s_off, s_ew = (a[order] for a in (s_qi, s_q, s_w, s_off, s_ew))
        blk_sizes = (T.reshape(-1) * 128)
        blk_base = np.concatenate([[0], np.cumsum(blk_sizes)])[:-1].reshape(NW, NQ)
        key = s_w * NQ + s_q
        grp_start = np.searchsorted(key, np.arange(NW * NQ), side="left")
        slot = blk_base[s_w, s_q] + (np.arange(key.size) - grp_start[key])

        nslots = ntiles * 128
        gidx = np.zeros(nslots, np.int16)
        ewv = np.zeros(nslots, np.float32)
        offv = np.full(nslots, -1, np.int64)
        gidx[slot] = s_qi.astype(np.int16)
        ewv[slot] = s_ew
        offv[slot] = s_off

        gw = np.tile(gidx.reshape(-1, 16).T, (8, 1))       # [128, nslots/16]

        offm = offv.reshape(ntiles, 128).T
        ewm = ewv.reshape(ntiles, 128).T
        sidx = offm.astype(np.int16)          # dst offset in window, -1 = pad
        sidx = np.pad(sidx, ((0, 0), (0, ntiles_pad - ntiles)), constant_values=-1)
        tmod = np.arange(ntiles_pad) % (LS_T // 2)
        sidx16 = np.where(sidx >= 0, tmod[None, :] * 128 + sidx, -1).astype(np.int16)
        ewm = np.pad(ewm, ((0, 0), (0, ntiles_pad - ntiles)))

        loc = newloc[nstart[c]: nstart[c + 1]]
        xT1 = np.zeros((NF + 1, NP), np.float32)
        xT1[:NF, loc] = x[nstart[c]: nstart[c + 1]].T
        xT1[NF, :] = 1.0

        nb = (batch[nstart[c]: nstart[c + 1]] - c * GPC).astype(np.int64)
        cnts = np.bincount(nb, minlength=GPC).astype(np.float32)
        pool = np.zeros((128, NP), np.float32)
        pool[loc & 127, (loc >> 7) * 128 + nb] = 1.0 / np.maximum(cnts[nb], 1.0)

        per_core.append(dict(gidx=gw, sidx=sidx, sidx16=sidx16, ew=ewm,
                             xT1=xT1, pool=pool, ngraphs=GPC))

    meta = dict(T=T, ntiles=ntiles, ntiles_pad=ntiles_pad,
                NW=NW, NQ=NQ, QROWS=QROWS, NF=NF, cfg=cfg)
    return per_core, meta


def weights_map(inputs):
    f32 = np.float32
    vs = np.vstack
    w = {"embWb": vs([np.asarray(inputs["emb_w"], f32),
                      np.asarray(inputs["emb_b"], f32)[None]]),
         "L1b": vs([np.asarray(inputs["lin1_w"], f32),
                    np.asarray(inputs["lin1_b"], f32)[None]]),
         "L2b": vs([np.asarray(inputs["lin2_w"], f32),
                    np.asarray(inputs["lin2_b"], f32)[None]]),
         "ident": np.eye(128, dtype=f32),
         "iotaG": np.tile(np.tile(np.arange(128, dtype=f32),
                          FULL_CFG["ls_t"]), (128, 1))}
    L = np.asarray(inputs["conv_w1"]).shape[0]
    for l in range(L):
        w[f"W1b_{l}"] = vs([np.asarray(inputs["conv_w1"][l], f32),
                            np.asarray(inputs["conv_b1"][l], f32)[None]])
        w[f"W2_{l}"] = np.asarray(inputs["conv_w2"][l], f32)
        w3 = vs([np.asarray(inputs["conv_w3"][l], f32),
                 np.asarray(inputs["conv_b3"][l], f32)[None]])
        w[f"W3b_{l}"] = np.hstack([w3, np.zeros((w3.shape[0], 1), f32)]) \
            if l == 0 else w3
    return w


# --------------------------------------------------------------- builder
def build_graph(meta):
    from concourse import bacc, mybir, tile

    cfg = meta["cfg"]
    NC, H, L = cfg["nc"], cfg["hid"], cfg["n_layers"]
    NP, NW, NQ, QROWS = cfg["node_pad"], meta["NW"], meta["NQ"], meta["QROWS"]
    NF = meta["NF"]
    H2 = 2 * H                                  # padded bf16 a-row (256B)
    T = meta["T"]
    ntiles, ntiles_pad = meta["ntiles"], meta["ntiles_pad"]
    LS_T = cfg["ls_t"]
    TMAXQ = int(T.max())
    TWMAX = int(T.sum(axis=1).max())
    f32, bf16, i16 = mybir.dt.float32, mybir.dt.bfloat16, mybir.dt.int16
    AF = mybir.ActivationFunctionType
    NCH = NP // 512

    nc = bacc.Bacc(num_devices=NC, num_swdge_queues=4)

    gidx_d = nc.declare_dram_parameter("gidx", [128, ntiles * 8], i16, False)
    sidx_d = nc.declare_dram_parameter("sidx", [128, ntiles_pad], bf16, False)
    sidx16_d = nc.declare_dram_parameter("sidx16", [128, ntiles_pad], i16, False)
    ew_d = nc.declare_dram_parameter("ew", [128, ntiles_pad], bf16, False)
    xT1_d = nc.declare_dram_parameter("xT1", [NF + 1, NP], bf16, False)
    pool_d = nc.declare_dram_parameter("pool", [128, NP], bf16, False)
    wnames = (["embWb", "L1b", "L2b", "ident", "iotaG"]
              + [f"{p}_{l}" for l in range(L) for p in ("W1b", "W2", "W3b")])
    wshape = {"embWb": [NF + 1, H], "L1b": [H + 1, H], "L2b": [H + 1, 3],
              "ident": [128, 128], "iotaG": [128, LS_T * 128]}
    wdt = {"embWb": bf16, "L1b": f32, "L2b": f32, "ident": f32, "iotaG": bf16}
    for l in range(L):
        wshape[f"W1b_{l}"] = [H + 1, H]
        wshape[f"W2_{l}"] = [H, H]
        wshape[f"W3b_{l}"] = [H + 1, H + 1] if l == 0 else [H + 1, H]
        wdt[f"W1b_{l}"] = wdt[f"W2_{l}"] = wdt[f"W3b_{l}"] = bf16
    wd = {k: nc.declare_dram_parameter(k, wshape[k], wdt[k], False)
          for k in wnames}
    out_d = nc.declare_dram_parameter("out", [128, 4], f32, True)

    with tile.TileContext(nc) as tc:
        with (
            tc.tile_pool(name="res", bufs=1) as res,
            tc.tile_pool(name="dram", bufs=1, space="DRAM") as dram,
            tc.tile_pool(name="stage", bufs=2) as stage,
            tc.tile_pool(name="gbuf", bufs=10) as gpool,
            tc.tile_pool(name="ohbuf", bufs=5) as ohpool,
            tc.tile_pool(name="ixbuf", bufs=3) as ixpool,
            tc.tile_pool(name="scr", bufs=2) as scr,
            tc.tile_pool(name="psA", bufs=2, space="PSUM") as psA,
            tc.tile_pool(name="psB", bufs=2, space="PSUM") as psB,
            tc.tile_pool(name="psC", bufs=3, space="PSUM") as psC,
            tc.tile_pool(name="psG", bufs=1, space="PSUM") as psG,
        ):
            a_loc_pp = [dram.tile([NP, H2], bf16, name=f"a_loc{i}")
                        for i in range(2)]
            a_full_pp = [dram.tile([NP * NC, H2], bf16, name=f"a_full{i}",
                                   addr_space="Shared")
                         for i in range(4)]
            h_nm_d = dram.tile([NW, 128, H], bf16)

            hT = res.tile([H + 1, NP], bf16, tag="hT")
            t2T = res.tile([H, NP], bf16, tag="t2T")
            aggT = res.tile([H + 1, NP], bf16, tag="aggT")
            dgw = res.tile([H, NP], bf16, tag="dgw")
            sidx_s = res.tile([128, ntiles_pad], bf16, tag="sidx")
            sidx16_s = res.tile([128, ntiles_pad], i16, tag="sidx16")
            ew_s = res.tile([128, ntiles_pad], bf16, tag="ew")
            ones_row = res.tile([1, H], bf16, tag="ones_row")
            wts = {k: res.tile(wshape[k], wdt[k], tag=k, name=k) for k in wnames}
            ident = wts["ident"]

            nc.vector.memset(ones_row[:], 1.0)
            for k in wnames:
                nc.sync.dma_start(wts[k][:], wd[k][:])
            nc.sync.dma_start(sidx_s[:], sidx_d[:])
            nc.sync.dma_start(sidx16_s[:], sidx16_d[:])
            nc.sync.dma_start(ew_s[:], ew_d[:])

            # h0T = embWb.T @ xT1 (streamed)
            for ck in range(NCH):
                sl = slice(ck * 512, (ck + 1) * 512)
                xc = stage.tile([NF + 1, 512], bf16, tag="xc")
                nc.sync.dma_start(xc[:], xT1_d[:, sl])
                ps = psA.tile([H, 512], f32, tag="dps")
                nc.tensor.matmul(ps[:], wts["embWb"][:], xc[:],
                                 start=True, stop=True)
                nc.scalar.activation(hT[:H, sl], ps[:], AF.Copy)
            nc.vector.memset(hT[H:H + 1, :], 1.0)

            for l in range(L):
                layer0 = l == 0
                HD = H + 1 if layer0 else H
                a_loc = a_loc_pp[l % 2]
                a_full = a_full_pp[l]

                # ---- dense: a (node-major, ones col at elem 64) -> a_loc; t2T
                for ck in range(NCH if not ABL_DENSE else 0):
                    sl = slice(ck * 512, (ck + 1) * 512)
                    ast = stage.tile([128, 4, H2], bf16, tag="ast")
                    nc.vector.memset(ast[:, :, H:], 0.0)
                    nc.vector.memset(ast[:, :, H:H + 1], 1.0)
                    for j in range(4):
                        wsl = slice(ck * 512 + j * 128, ck * 512 + (j + 1) * 128)
                        pst = psB.tile([128, H], f32, tag="tps")
                        nc.tensor.matmul(pst[:], hT[:, wsl], wts[f"W1b_{l}"][:],
                                         start=True, stop=True)
                        nc.vector.tensor_copy(ast[:, j, :H], pst[:])
                    nc.sync.dma_start(
                        a_loc[sl, :].rearrange("(j p) f -> p j f", p=128), ast[:])
                    ps2 = psA.tile([H, 512], f32, tag="dps")
                    nc.tensor.matmul(ps2[:], wts[f"W2_{l}"][:], hT[:H, sl],
                                     start=True, stop=True)
                    nc.scalar.activation(t2T[:, sl], ps2[:], AF.Copy)

                if not ABL_AG:
                    nc.gpsimd.collective_compute(
                        "AllGather", mybir.AluOpType.bypass,
                        replica_groups=[list(range(NC))],
                        ins=[a_loc[:].opt()], outs=[a_full[:].opt()])

                # ---- window-major gather+scatter stream
                t = 0
                gcall = 0
                ohb = None
                for w in range(NW):
                    wsl = slice(w * 128, (w + 1) * 128)
                    Tw = int(T[w].sum())
                    if not ABL_FLUSH:
                        ps = psC.tile([H + 1, 128], f32, tag="sps")
                        nc.tensor.matmul(ps[:HD, :], wts[f"W3b_{l}"][:],
                                         hT[:, wsl], start=True, stop=False)
                    if not ABL_GIX:
                        gix = ixpool.tile([128, TWMAX * 8], i16, tag="gix")
                        nc.sync.dma_start(gix[:, :Tw * 8],
                                          gidx_d[:, t * 8:(t + Tw) * 8])
                    toff = 0
                    for q in range(NQ):
                        Twq = int(T[w][q])
                        gb = gpool.tile([128, TMAXQ, H2], bf16, tag="gb")
                        if ABL_GATHER:
                            nc.vector.memset(gb[:, :Twq, :], 0.01)
                        else:
                            nc.gpsimd.dma_gather(
                                gb[:, :Twq, :],
                                a_full[q * QROWS:(q + 1) * QROWS, :],
                                gix[:, toff * 8:(toff + Twq) * 8],
                                Twq * 128, Twq * 128, H2,
                                single_packet=False, queue_num=gcall % 4)
                            gcall += 1
                        for i in range(Twq):
                            if t % LS_T == 0:
                                ohb = ohpool.tile([128, LS_T * 128], bf16,
                                                  tag="ohb")
                                g0 = t
                                if ABL_LSCAT:
                                    nc.vector.memset(ohb[:], 0.001)
                                elif (t // LS_T) % 5 == 4:
                                    HT_ = LS_T // 2
                                    for hh in range(2):
                                        nc.gpsimd.local_scatter(
                                            ohb[:, hh * HT_ * 128:
                                                (hh + 1) * HT_ * 128],
                                            ew_s[:, g0 + hh * HT_:
                                                 g0 + (hh + 1) * HT_],
                                            sidx16_s[:, g0 + hh * HT_:
                                                     g0 + (hh + 1) * HT_],
                                            channels=128, num_elems=HT_ * 128,
                                            num_idxs=HT_)
                                else:
                                    oh3 = ohb[:].rearrange(
                                        "p (t c) -> p t c", c=128)
                                    sib = (sidx_s[:, g0:g0 + LS_T]
                                           .rearrange("p (t o) -> p t o", o=1)
                                           .broadcast_to([128, LS_T, 128]))
                                    ewb = (ew_s[:, g0:g0 + LS_T]
                                           .rearrange("p (t o) -> p t o", o=1)
                                           .broadcast_to([128, LS_T, 128]))
                                    it3 = wts["iotaG"][:].rearrange(
                                        "p (t c) -> p t c", c=128)
                                    nc.vector.tensor_tensor(
                                        oh3, it3, sib, mybir.AluOpType.is_equal)
                                    nc.vector.tensor_tensor(
                                        oh3, oh3, ewb, mybir.AluOpType.mult)
                            oh_sl = ohb[:, (t % LS_T) * 128:(t % LS_T + 1) * 128]
                            last = (q == NQ - 1) and (i == Twq - 1)
                            if not ABL_FLUSH and (not ABL_MM or last):
                                nc.tensor.matmul(
                                    ps[:HD, :], gb[:, i, :HD], oh_sl,
                                    start=False, stop=last)
                            t += 1
                            toff += 1
                    if not ABL_FLUSH:
                        nc.scalar.activation(aggT[:HD, wsl], ps[:HD, :], AF.Copy)

                if layer0 and not ABL_COMB:
                    # dgw = PE-broadcast of deg row (via partition-0 staging)
                    for ck in range(NCH):
                        sl = slice(ck * 512, (ck + 1) * 512)
                        dr = stage.tile([1, 512], bf16, tag="dr")
                        nc.sync.dma_start(dr[:], aggT[H:H + 1, sl])
                        psr = psA.tile([H, 512], f32, tag="dps")
                        nc.tensor.matmul(psr[:], ones_row[:], dr[:],
                                         start=True, stop=True)
                        nc.scalar.activation(dgw[:, sl], psr[:], AF.Copy)

                # ---- combine: h = relu(aggT - t2T*dgw)
                if ABL_COMB:
                    continue
                if l < L - 1:
                    for ck in range(NCH):
                        csl = slice(ck * 512, (ck + 1) * 512)
                        tmp = scr.tile([H, 512], f32, tag="cmb1")
                        nc.vector.tensor_mul(tmp[:], t2T[:, csl], dgw[:, csl])
                        nc.vector.tensor_sub(tmp[:], aggT[:H, csl], tmp[:])
                        nc.scalar.activation(hT[:H, csl], tmp[:], AF.Relu)
                else:
                    for w in range(NW):
                        wsl = slice(w * 128, (w + 1) * 128)
                        tmp = scr.tile([H, 128], f32, tag="cmb2")
                        nc.vector.tensor_mul(tmp[:], t2T[:, wsl], dgw[:, wsl])
                        nc.vector.tensor_sub(tmp[:], aggT[:H, wsl], tmp[:])
                        hTw = scr.tile([H, 128], f32, tag="cmb3")
                        nc.scalar.activation(hTw[:], tmp[:], AF.Relu)
                        pst = psB.tile([128, H], f32, tag="tps")
                        nc.tensor.transpose(pst[:], hTw[:], ident[:H, :H])
                        hst = scr.tile([128, H], bf16, tag="hst")
                        nc.vector.tensor_copy(hst[:], pst[:])
                        nc.sync.dma_start(h_nm_d[w], hst[:])

            # ---- pooling + MLP
            if ABL_TAIL:
                outs0 = stage.tile([128, 4], f32, tag="outs")
                nc.vector.memset(outs0[:], 0.0)
                nc.sync.dma_start(out_d[:], outs0[:])
            else:
                psg = psG.tile([128, H], f32, tag="spsg")
                for w in range(NW):
                    pw = ixpool.tile([128, 128], bf16, tag="pw")
                    nc.sync.dma_start(pw[:], pool_d[:, w * 128:(w + 1) * 128])
                    hb = ixpool.tile([128, H], bf16, tag="hb")
                    nc.sync.dma_start(hb[:], h_nm_d[w])
                    nc.tensor.matmul(psg[:], pw[:], hb[:],
                                     start=(w == 0), stop=(w == NW - 1))
                gx = stage.tile([128, H], f32, tag="gx")
                nc.vector.tensor_copy(gx[:], psg[:])
                pst = psB.tile([128, 128], f32, tag="tps")
                nc.tensor.transpose(pst[:H, :], gx[:], ident[:])
                gxT = stage.tile([H + 1, 128], f32, tag="gxT")
                nc.vector.tensor_copy(gxT[:H, :], pst[:H, :])
                nc.vector.memset(gxT[H:H + 1, :], 1.0)
                ps1 = psB.tile([128, H], f32, tag="tps")
                nc.tensor.matmul(ps1[:], gxT[:], wts["L1b"][:],
                                 start=True, stop=True)
                r1 = stage.tile([128, H], f32, tag="r1")
                nc.scalar.activation(r1[:], ps1[:], AF.Relu)
                pst2 = psB.tile([128, 128], f32, tag="tps")
                nc.tensor.transpose(pst2[:H, :], r1[:], ident[:])
                r1T = stage.tile([H + 1, 128], f32, tag="r1T")
                nc.vector.tensor_copy(r1T[:H, :], pst2[:H, :])
                nc.vector.memset(r1T[H:H + 1, :], 1.0)
                ps2 = psB.tile([128, 4], f32, tag="tps")
                nc.tensor.matmul(ps2[:, :3], r1T[:], wts["L2b"][:],
                                 start=True, stop=True)
                outs = stage.tile([128, 4], f32, tag="outs")
                nc.vector.memset(outs[:], 0.0)
                nc.vector.tensor_copy(outs[:, :3], ps2[:, :3])
                nc.sync.dma_start(out_d[:], outs[:])

    nc.compile()
    return nc


# --------------------------------------------------------------- entry
F32_KEYS = ("L1b", "L2b", "ident")


def make_in_maps(per_core, w, cfg):
    import ml_dtypes
    bf = ml_dtypes.bfloat16
    in_maps = []
    for c in range(cfg["nc"]):
        pc = per_core[c]
        m = {}
        for k, v in w.items():
            m[k] = v if k in F32_KEYS else v.astype(bf)
        m["gidx"], m["sidx"] = pc["gidx"], pc["sidx"].astype(bf)
        m["sidx16"] = pc["sidx16"]
        m["ew"] = pc["ew"].astype(bf)
        m["xT1"] = pc["xT1"].astype(bf)
        m["pool"] = pc["pool"].astype(bf)
        in_maps.append(m)
    return in_maps


def run(inputs, cfg, trace=False):
    per_core, meta = shard_and_pack(inputs, cfg)
    w = weights_map(inputs)
    in_maps = make_in_maps(per_core, w, cfg)
    nc = build_graph(meta)
    from concourse import bass_utils
    res = bass_utils.run_bass_kernel_spmd(
        nc, in_maps, core_ids=list(range(cfg["nc"])), trace=trace)
    outs = [np.asarray(res.results[c]["out"])[:per_core[c]["ngraphs"], :3]
            for c in range(cfg["nc"])]
    return np.concatenate(outs, 0).astype(np.float32), res


def kernel(**inputs):
    out, _ = run(inputs, FULL_CFG)
    return out


# revision 60
# speedup vs baseline: 1.3441x; 1.3335x over previous
"""Trainium2 Bass kernel for BA3MotifNet (4-layer LEConv GNN + mean-pool + MLP).

SPMD across 8 NeuronCores, single compiled graph; all per-core variation is
carried in the input data (index streams), never in instruction structure.

  - Nodes dst-sharded at graph boundaries (batch sorted): core c owns graphs
    [125c,125(c+1)) and their nodes, padded to NODE_PAD=12800/core.
  - Per layer: a = h@W1+b1 computed node-major (nodes in PSUM partition dim),
    a ones-column rides element 64 of each 256B bf16 row; DMA'd to DRAM,
    AllGather -> a_full [102400,128] (Shared, one buffer per layer).
  - agg_i = sum_{e:dst=i} ew_e*a[src_e] - (h@W2)_i * degw_i.
    Tile stream is WINDOW-MAJOR: for each 128-dst window, the 4 source
    quarters' tiles run consecutively, accumulating in one PSUM region;
    a single scalar-engine Copy flushes agg (+deg row on layer 0) to aggT.
    Gather: one SWDGE dma_gather per (window, quarter) run of 256B rows from
    a_full, round-robin across 4 SWDGE queues; int16 indices.
    Scatter: PE matmul aggT[f,d] += gathered[e,f].T @ onehot[e,d]; onehot
    [128, LS_T*128] groups built on DVE by broadcast tensor_tensor
    (iota==sidx)*ew -- no PSUM-coupled DVE ops anywhere in the stream.
  - h = relu(aggT - t2T*dgw) on 512-col slabs; layer 3 produces node-major
    bf16 h staged via DRAM for pooling.
  - Mean-pool via (1/cnt)-valued one-hot matmuls; 2-layer MLP on-core.
  - Out: per-core [128,4] f32 -> host concat -> [1000,3].
"""

import os
import sys

import numpy as np

sys.path.insert(0, "/opt/trn_rl_repo")

ABL_GATHER = os.environ.get("ABL_GATHER", "0") == "1"   # memset instead of gather
ABL_LSCAT = os.environ.get("ABL_LSCAT", "0") == "1"     # memset instead of onehot
ABL_AG = os.environ.get("ABL_AG", "0") == "1"           # skip AllGather collective
ABL_MM = os.environ.get("ABL_MM", "0") == "1"           # skip per-tile matmuls
ABL_DENSE = os.environ.get("ABL_DENSE", "0") == "1"     # skip dense a/t2 phase
ABL_COMB = os.environ.get("ABL_COMB", "0") == "1"       # skip combine + dgw
ABL_TAIL = os.environ.get("ABL_TAIL", "0") == "1"       # skip pooling + MLP
ABL_FLUSH = os.environ.get("ABL_FLUSH", "0") == "1"     # skip W3b/flush/psC
ABL_GIX = os.environ.get("ABL_GIX", "0") == "1"         # skip gix loads

FULL_CFG = dict(
    n_nodes=100000, n_edges=3200000, n_graphs=1000, hid=64, n_layers=4,
    nc=8, node_pad=12800, ls_t=28,
)


# --------------------------------------------------------------- host prep
def shard_and_pack(inputs, cfg):
    NC, NP = cfg["nc"], cfg["node_pad"]
    NW, NQ = NP // 128, 4
    QROWS = NP * NC // NQ
    G = cfg["n_graphs"]
    GPC = G // NC
    assert QROWS <= 32768

    x = np.asarray(inputs["x"], np.float32)
    ei = np.asarray(inputs["edge_index"], np.int64)
    ew = np.asarray(inputs["edge_attr"], np.float32)
    batch = np.asarray(inputs["batch"], np.int64)
    N = x.shape[0]
    NF = x.shape[1]

    gs = np.searchsorted(batch, np.arange(G + 1))
    nstart = gs[np.arange(NC + 1) * GPC]
    ncnt = np.diff(nstart)
    if ncnt.max() > NP:                                    # rare: grow pad
        NP = int(-(-int(ncnt.max()) // 512) * 512)
        cfg = dict(cfg, node_pad=NP)
        NW = NP // 128
        QROWS = NP * NC // NQ
        assert QROWS <= 32768

    shard_of = np.searchsorted(nstart[1:], np.arange(N), side="right")
    src, dst = ei[0], ei[1]
    e_core = shard_of[dst]
    # quarter of a src node depends only on its shard (QROWS == 2*NP)
    e_q = shard_of[src] * NP // QROWS

    # degree-balanced window packing per core: relabel local node ids so every
    # (window, quarter) edge count is as even as possible.
    newloc = np.zeros(N, np.int64)
    for c in range(NC):
        n_c = int(ncnt[c])
        deg4 = np.zeros((NP, NQ), np.int64)
        selc = e_core == c
        np.add.at(deg4, (dst[selc] - nstart[c], e_q[selc]), 1)
        deg4 = deg4[:n_c]
        order = np.argsort(-deg4.sum(1), kind="stable")
        loads = np.zeros((NW, NQ), np.int64)
        fill = np.zeros(NW, np.int64)
        assign = np.zeros(n_c, np.int64)
        for n in order:
            new_loads = loads + deg4[n]
            over = np.maximum(0, new_loads - 1016).sum(1)
            cand = over * 1e6 + new_loads.max(1).astype(np.float64)
            cand[fill >= 128] = np.inf
            wsel = int(np.argmin(cand))
            assign[n] = wsel * 128 + fill[wsel]
            fill[wsel] += 1
            loads[wsel] += deg4[n]
        newloc[nstart[c]: nstart[c] + n_c] = assign

    spad = shard_of * NP + newloc
    dstloc = newloc[dst]
    e_w = dstloc >> 7

    cnt = np.zeros((NC, NW, NQ), np.int64)
    np.add.at(cnt, (e_core, e_w, e_q), 1)
    T = np.maximum(1, -(-cnt.max(axis=0) // 128))          # [NW, NQ]

    ntiles = int(T.sum())
    LS_T = cfg["ls_t"]
    ntiles_pad = -(-ntiles // LS_T) * LS_T

    per_core = []
    for c in range(NC):
        sel = e_core == c
        s_qi = (spad[src[sel]] % QROWS).astype(np.int64)
        s_q, s_w = e_q[sel], e_w[sel]
        s_off = (dstloc[sel] & 127).astype(np.int64)
        s_ew = ew[sel]

        # window-major slot blocks: tiles ordered (w, q)
        order = np.lexsort((s_off, s_q, s_w))
        s_qi, s_q, s_w, s_off, s_ew = (a[order] for a in (s_qi, s_q, s_w, s_off, s_ew))
        blk_sizes = (T.reshape(-1) * 128)
        blk_base = np.concatenate([[0], np.cumsum(blk_sizes)])[:-1].reshape(NW, NQ)
        key = s_w * NQ + s_q
        grp_start = np.searchsorted(key, np.arange(NW * NQ), side="left")
        slot = blk_base[s_w, s_q] + (np.arange(key.size) - grp_start[key])

        nslots = ntiles * 128
        gidx = np.zeros(nslots, np.int16)
        ewv = np.zeros(nslots, np.float32)
        offv = np.full(nslots, -1, np.int64)
        gidx[slot] = s_qi.astype(np.int16)
        ewv[slot] = s_ew
        offv[slot] = s_off

        gw = np.tile(gidx.reshape(-1, 16).T, (8, 1))       # [128, nslots/16]

        offm = offv.reshape(ntiles, 128).T
        ewm = ewv.reshape(ntiles, 128).T
        sidx = offm.astype(np.int16)          # dst offset in window, -1 = pad
        sidx = np.pad(sidx, ((0, 0), (0, ntiles_pad - ntiles)), constant_values=-1)
        ewm = np.pad(ewm, ((0, 0), (0, ntiles_pad - ntiles)))

        loc = newloc[nstart[c]: nstart[c + 1]]
        xT1 = np.zeros((NF + 1, NP), np.float32)
        xT1[:NF, loc] = x[nstart[c]: nstart[c + 1]].T
        xT1[NF, :] = 1.0

        nb = (batch[nstart[c]: nstart[c + 1]] - c * GPC).astype(np.int64)
        cnts = np.bincount(nb, minlength=GPC).astype(np.float32)
        pool = np.zeros((128, NP), np.float32)
        pool[loc & 127, (loc >> 7) * 128 + nb] = 1.0 / np.maximum(cnts[nb], 1.0)

        per_core.append(dict(gidx=gw, sidx=sidx, ew=ewm, xT1=xT1, pool=pool,
                             ngraphs=GPC))

    meta = dict(T=T, ntiles=ntiles, ntiles_pad=ntiles_pad,
                NW=NW, NQ=NQ, QROWS=QROWS, NF=NF, cfg=cfg)
    return per_core, meta


def weights_map(inputs):
    f32 = np.float32
    vs = np.vstack
    w = {"embWb": vs([np.asarray(inputs["emb_w"], f32),
                      np.asarray(inputs["emb_b"], f32)[None]]),
         "L1b": vs([np.asarray(inputs["lin1_w"], f32),
                    np.asarray(inputs["lin1_b"], f32)[None]]),
         "L2b": vs([np.asarray(inputs["lin2_w"], f32),
                    np.asarray(inputs["lin2_b"], f32)[None]]),
         "ident": np.eye(128, dtype=f32),
         "iotaG": np.tile(np.tile(np.arange(128, dtype=f32),
                          FULL_CFG["ls_t"]), (128, 1))}
    L = np.asarray(inputs["conv_w1"]).shape[0]
    for l in range(L):
        w[f"W1b_{l}"] = vs([np.asarray(inputs["conv_w1"][l], f32),
                            np.asarray(inputs["conv_b1"][l], f32)[None]])
        w[f"W2_{l}"] = np.asarray(inputs["conv_w2"][l], f32)
        w3 = vs([np.asarray(inputs["conv_w3"][l], f32),
                 np.asarray(inputs["conv_b3"][l], f32)[None]])
        w[f"W3b_{l}"] = np.hstack([w3, np.zeros((w3.shape[0], 1), f32)]) \
            if l == 0 else w3
    return w


# --------------------------------------------------------------- builder
def build_graph(meta):
    from concourse import bacc, mybir, tile

    cfg = meta["cfg"]
    NC, H, L = cfg["nc"], cfg["hid"], cfg["n_layers"]
    NP, NW, NQ, QROWS = cfg["node_pad"], meta["NW"], meta["NQ"], meta["QROWS"]
    NF = meta["NF"]
    H2 = 2 * H                                  # padded bf16 a-row (256B)
    T = meta["T"]
    ntiles, ntiles_pad = meta["ntiles"], meta["ntiles_pad"]
    LS_T = cfg["ls_t"]
    TMAXQ = int(T.max())
    TWMAX = int(T.sum(axis=1).max())
    f32, bf16, i16 = mybir.dt.float32, mybir.dt.bfloat16, mybir.dt.int16
    AF = mybir.ActivationFunctionType
    NCH = NP // 512

    nc = bacc.Bacc(num_devices=NC, num_swdge_queues=4)

    gidx_d = nc.declare_dram_parameter("gidx", [128, ntiles * 8], i16, False)
    sidx_d = nc.declare_dram_parameter("sidx", [128, ntiles_pad], bf16, False)
    ew_d = nc.declare_dram_parameter("ew", [128, ntiles_pad], bf16, False)
    xT1_d = nc.declare_dram_parameter("xT1", [NF + 1, NP], bf16, False)
    pool_d = nc.declare_dram_parameter("pool", [128, NP], bf16, False)
    wnames = (["embWb", "L1b", "L2b", "ident", "iotaG"]
              + [f"{p}_{l}" for l in range(L) for p in ("W1b", "W2", "W3b")])
    wshape = {"embWb": [NF + 1, H], "L1b": [H + 1, H], "L2b": [H + 1, 3],
              "ident": [128, 128], "iotaG": [128, LS_T * 128]}
    wdt = {"embWb": bf16, "L1b": f32, "L2b": f32, "ident": f32, "iotaG": bf16}
    for l in range(L):
        wshape[f"W1b_{l}"] = [H + 1, H]
        wshape[f"W2_{l}"] = [H, H]
        wshape[f"W3b_{l}"] = [H + 1, H + 1] if l == 0 else [H + 1, H]
        wdt[f"W1b_{l}"] = wdt[f"W2_{l}"] = wdt[f"W3b_{l}"] = bf16
    wd = {k: nc.declare_dram_parameter(k, wshape[k], wdt[k], False)
          for k in wnames}
    out_d = nc.declare_dram_parameter("out", [128, 4], f32, True)

    with tile.TileContext(nc) as tc:
        with (
            tc.tile_pool(name="res", bufs=1) as res,
            tc.tile_pool(name="dram", bufs=1, space="DRAM") as dram,
            tc.tile_pool(name="stage", bufs=2) as stage,
            tc.tile_pool(name="gbuf", bufs=10) as gpool,
            tc.tile_pool(name="ohbuf", bufs=6) as ohpool,
            tc.tile_pool(name="ixbuf", bufs=3) as ixpool,
            tc.tile_pool(name="scr", bufs=2) as scr,
            tc.tile_pool(name="psA", bufs=2, space="PSUM") as psA,
            tc.tile_pool(name="psB", bufs=2, space="PSUM") as psB,
            tc.tile_pool(name="psC", bufs=3, space="PSUM") as psC,
            tc.tile_pool(name="psG", bufs=1, space="PSUM") as psG,
        ):
            a_loc_pp = [dram.tile([NP, H2], bf16, name=f"a_loc{i}")
                        for i in range(2)]
            a_full_pp = [dram.tile([NP * NC, H2], bf16, name=f"a_full{i}",
                                   addr_space="Shared")
                         for i in range(4)]
            h_nm_d = dram.tile([NW, 128, H], bf16)

            hT = res.tile([H + 1, NP], bf16, tag="hT")
            t2T = res.tile([H, NP], bf16, tag="t2T")
            aggT = res.tile([H + 1, NP], bf16, tag="aggT")
            dgw = res.tile([H, NP], bf16, tag="dgw")
            sidx_s = res.tile([128, ntiles_pad], bf16, tag="sidx")
            ew_s = res.tile([128, ntiles_pad], bf16, tag="ew")
            ones_row = res.tile([1, H], bf16, tag="ones_row")
            wts = {k: res.tile(wshape[k], wdt[k], tag=k, name=k) for k in wnames}
            ident = wts["ident"]

            nc.vector.memset(ones_row[:], 1.0)
            for k in wnames:
                nc.sync.dma_start(wts[k][:], wd[k][:])
            nc.sync.dma_start(sidx_s[:], sidx_d[:])
            nc.sync.dma_start(ew_s[:], ew_d[:])

            # h0T = embWb.T @ xT1 (streamed)
            for ck in range(NCH):
                sl = slice(ck * 512, (ck + 1) * 512)
                xc = stage.tile([NF + 1, 512], bf16, tag="xc")
                nc.sync.dma_start(xc[:], xT1_d[:, sl])
                ps = psA.tile([H, 512], f32, tag="dps")
                nc.tensor.matmul(ps[:], wts["embWb"][:], xc[:],
                                 start=True, stop=True)
                nc.scalar.activation(hT[:H, sl], ps[:], AF.Copy)
            nc.vector.memset(hT[H:H + 1, :], 1.0)

            for l in range(L):
                layer0 = l == 0
                HD = H + 1 if layer0 else H
                a_loc = a_loc_pp[l % 2]
                a_full = a_full_pp[l]

                # ---- dense: a (node-major, ones col at elem 64) -> a_loc; t2T
                for ck in range(NCH if not ABL_DENSE else 0):
                    sl = slice(ck * 512, (ck + 1) * 512)
                    ast = stage.tile([128, 4, H2], bf16, tag="ast")
                    nc.vector.memset(ast[:, :, H:], 0.0)
                    nc.vector.memset(ast[:, :, H:H + 1], 1.0)
                    for j in range(4):
                        wsl = slice(ck * 512 + j * 128, ck * 512 + (j + 1) * 128)
                        pst = psB.tile([128, H], f32, tag="tps")
                        nc.tensor.matmul(pst[:], hT[:, wsl], wts[f"W1b_{l}"][:],
                                         start=True, stop=True)
                        nc.vector.tensor_copy(ast[:, j, :H], pst[:])
                    nc.sync.dma_start(
                        a_loc[sl, :].rearrange("(j p) f -> p j f", p=128), ast[:])
                    ps2 = psA.tile([H, 512], f32, tag="dps")
                    nc.tensor.matmul(ps2[:], wts[f"W2_{l}"][:], hT[:H, sl],
                                     start=True, stop=True)
                    nc.scalar.activation(t2T[:, sl], ps2[:], AF.Copy)

                if not ABL_AG:
                    nc.gpsimd.collective_compute(
                        "AllGather", mybir.AluOpType.bypass,
                        replica_groups=[list(range(NC))],
                        ins=[a_loc[:].opt()], outs=[a_full[:].opt()])

                # ---- window-major gather+scatter stream
                t = 0
                gcall = 0
                ohb = None
                for w in range(NW):
                    wsl = slice(w * 128, (w + 1) * 128)
                    Tw = int(T[w].sum())
                    if not ABL_FLUSH:
                        ps = psC.tile([H + 1, 128], f32, tag="sps")
                        nc.tensor.matmul(ps[:HD, :], wts[f"W3b_{l}"][:],
                                         hT[:, wsl], start=True, stop=False)
                    if not ABL_GIX:
                        gix = ixpool.tile([128, TWMAX * 8], i16, tag="gix")
                        nc.sync.dma_start(gix[:, :Tw * 8],
                                          gidx_d[:, t * 8:(t + Tw) * 8])
                    toff = 0
                    for q in range(NQ):
                        Twq = int(T[w][q])
                        gb = gpool.tile([128, TMAXQ, H2], bf16, tag="gb")
                        if ABL_GATHER:
                            nc.vector.memset(gb[:, :Twq, :], 0.01)
                        else:
                            nc.gpsimd.dma_gather(
                                gb[:, :Twq, :],
                                a_full[q * QROWS:(q + 1) * QROWS, :],
                                gix[:, toff * 8:(toff + Twq) * 8],
                                Twq * 128, Twq * 128, H2,
                                single_packet=False, queue_num=gcall % 4)
                            gcall += 1
                        for i in range(Twq):
                            if t % LS_T == 0:
                                ohb = ohpool.tile([128, LS_T * 128], bf16,
                                                  tag="ohb")
                                g0 = t
                                if ABL_LSCAT:
                                    nc.vector.memset(ohb[:], 0.001)
                                else:
                                    oh3 = ohb[:].rearrange(
                                        "p (t c) -> p t c", c=128)
                                    sib = (sidx_s[:, g0:g0 + LS_T]
                                           .rearrange("p (t o) -> p t o", o=1)
                                           .broadcast_to([128, LS_T, 128]))
                                    ewb = (ew_s[:, g0:g0 + LS_T]
                                           .rearrange("p (t o) -> p t o", o=1)
                                           .broadcast_to([128, LS_T, 128]))
                                    it3 = wts["iotaG"][:].rearrange(
                                        "p (t c) -> p t c", c=128)
                                    nc.vector.tensor_tensor(
                                        oh3, it3, sib, mybir.AluOpType.is_equal)
                                    nc.vector.tensor_tensor(
                                        oh3, oh3, ewb, mybir.AluOpType.mult)
                            oh_sl = ohb[:, (t % LS_T) * 128:(t % LS_T + 1) * 128]
                            last = (q == NQ - 1) and (i == Twq - 1)
                            if not ABL_FLUSH and (not ABL_MM or last):
                                nc.tensor.matmul(
                                    ps[:HD, :], gb[:, i, :HD], oh_sl,
                                    start=False, stop=last)
                            t += 1
                            toff += 1
                    if not ABL_FLUSH:
                        nc.scalar.activation(aggT[:HD, wsl], ps[:HD, :], AF.Copy)

                if layer0 and not ABL_COMB:
                    # dgw = PE-broadcast of deg row (via partition-0 staging)
                    for ck in range(NCH):
                        sl = slice(ck * 512, (ck + 1) * 512)
                        dr = stage.tile([1, 512], bf16, tag="dr")
                        nc.sync.dma_start(dr[:], aggT[H:H + 1, sl])
                        psr = psA.tile([H, 512], f32, tag="dps")
                        nc.tensor.matmul(psr[:], ones_row[:], dr[:],
                                         start=True, stop=True)
                        nc.scalar.activation(dgw[:, sl], psr[:], AF.Copy)

                # ---- combine: h = relu(aggT - t2T*dgw)
                if ABL_COMB:
                    continue
                if l < L - 1:
                    for ck in range(NCH):
                        csl = slice(ck * 512, (ck + 1) * 512)
                        tmp = scr.tile([H, 512], f32, tag="cmb1")
                        nc.vector.tensor_mul(tmp[:], t2T[:, csl], dgw[:, csl])
                        nc.vector.tensor_sub(tmp[:], aggT[:H, csl], tmp[:])
                        nc.scalar.activation(hT[:H, csl], tmp[:], AF.Relu)
                else:
                    for w in range(NW):
                        wsl = slice(w * 128, (w + 1) * 128)
                        tmp = scr.tile([H, 128], f32, tag="cmb2")
                        nc.vector.tensor_mul(tmp[:], t2T[:, wsl], dgw[:, wsl])
                        nc.vector.tensor_sub(tmp[:], aggT[:H, wsl], tmp[:])
                        hTw = scr.tile([H, 128], f32, tag="cmb3")
                        nc.scalar.activation(hTw[:], tmp[:], AF.Relu)
                        pst = psB.tile([128, H], f32, tag="tps")
                        nc.tensor.transpose(pst[:], hTw[:], ident[:H, :H])
                        hst = scr.tile([128, H], bf16, tag="hst")
                        nc.vector.tensor_copy(hst[:], pst[:])
                        nc.sync.dma_start(h_nm_d[w], hst[:])

            # ---- pooling + MLP
            if ABL_TAIL:
                outs0 = stage.tile([128, 4], f32, tag="outs")
                nc.vector.memset(outs0[:], 0.0)
                nc.sync.dma_start(out_d[:], outs0[:])
            else:
                psg = psG.tile([128, H], f32, tag="spsg")
                for w in range(NW):
                    pw = ixpool.tile([128, 128], bf16, tag="pw")
                    nc.sync.dma_start(pw[:], pool_d[:, w * 128:(w + 1) * 128])
                    hb = ixpool.tile([128, H], bf16, tag="hb")
                    nc.sync.dma_start(hb[:], h_nm_d[w])
                    nc.tensor.matmul(psg[:], pw[:], hb[:],
                                     start=(w == 0), stop=(w == NW - 1))
                gx = stage.tile([128, H], f32, tag="gx")
                nc.vector.tensor_copy(gx[:], psg[:])
                pst = psB.tile([128, 128], f32, tag="tps")
                nc.tensor.transpose(pst[:H, :], gx[:], ident[:])
                gxT = stage.tile([H + 1, 128], f32, tag="gxT")
                nc.vector.tensor_copy(gxT[:H, :], pst[:H, :])
                nc.vector.memset(gxT[H:H + 1, :], 1.0)
                ps1 = psB.tile([128, H], f32, tag="tps")
                nc.tensor.matmul(ps1[:], gxT[:], wts["L1b"][:],
                                 start=True, stop=True)
                r1 = stage.tile([128, H], f32, tag="r1")
                nc.scalar.activation(r1[:], ps1[:], AF.Relu)
                pst2 = psB.tile([128, 128], f32, tag="tps")
                nc.tensor.transpose(pst2[:H, :], r1[:], ident[:])
                r1T = stage.tile([H + 1, 128], f32, tag="r1T")
                nc.vector.tensor_copy(r1T[:H, :], pst2[:H, :])
                nc.vector.memset(r1T[H:H + 1, :], 1.0)
                ps2 = psB.tile([128, 4], f32, tag="tps")
                nc.tensor.matmul(ps2[:, :3], r1T[:], wts["L2b"][:],
                                 start=True, stop=True)
                outs = stage.tile([128, 4], f32, tag="outs")
                nc.vector.memset(outs[:], 0.0)
                nc.vector.tensor_copy(outs[:, :3], ps2[:, :3])
                nc.sync.dma_start(out_d[:], outs[:])

    nc.compile()
    return nc


# --------------------------------------------------------------- entry
F32_KEYS = ("L1b", "L2b", "ident")


def make_in_maps(per_core, w, cfg):
    import ml_dtypes
    bf = ml_dtypes.bfloat16
    in_maps = []
    for c in range(cfg["nc"]):
        pc = per_core[c]
        m = {}
        for k, v in w.items():
            m[k] = v if k in F32_KEYS else v.astype(bf)
        m["gidx"], m["sidx"] = pc["gidx"], pc["sidx"].astype(bf)
        m["ew"] = pc["ew"].astype(bf)
        m["xT1"] = pc["xT1"].astype(bf)
        m["pool"] = pc["pool"].astype(bf)
        in_maps.append(m)
    return in_maps


def run(inputs, cfg, trace=False):
    per_core, meta = shard_and_pack(inputs, cfg)
    w = weights_map(inputs)
    in_maps = make_in_maps(per_core, w, cfg)
    nc = build_graph(meta)
    from concourse import bass_utils
    res = bass_utils.run_bass_kernel_spmd(
        nc, in_maps, core_ids=list(range(cfg["nc"])), trace=trace)
    outs = [np.asarray(res.results[c]["out"])[:per_core[c]["ngraphs"], :3]
            for c in range(cfg["nc"])]
    return np.concatenate(outs, 0).astype(np.float32), res


def kernel(**inputs):
    out, _ = run(inputs, FULL_CFG)
    return out


# revision 61
# speedup vs baseline: 1.4570x; 1.0840x over previous
"""Trainium2 Bass kernel for BA3MotifNet (4-layer LEConv GNN + mean-pool + MLP).

SPMD across 8 NeuronCores, single compiled graph; all per-core variation is
carried in the input data (index streams), never in instruction structure.

  - Nodes dst-sharded at graph boundaries (batch sorted): core c owns graphs
    [125c,125(c+1)) and their nodes, padded to NODE_PAD=12800/core.
  - Per layer: a = h@W1+b1 computed node-major (nodes in PSUM partition dim),
    a ones-column rides element 64 of each 256B bf16 row; DMA'd to DRAM,
    AllGather -> a_full [102400,128] (Shared, one buffer per layer).
  - agg_i = sum_{e:dst=i} ew_e*a[src_e] - (h@W2)_i * degw_i.
    Tile stream is WINDOW-MAJOR: for each 128-dst window, the 4 source
    quarters' tiles run consecutively, accumulating in one PSUM region;
    a single scalar-engine Copy flushes agg (+deg row on layer 0) to aggT.
    Gather: one SWDGE dma_gather per (window, quarter) run of 256B rows from
    a_full, round-robin across 4 SWDGE queues; int16 indices.
    Scatter: PE matmul aggT[f,d] += gathered[e,f].T @ onehot[e,d]; onehot
    [128, LS_T*128] groups built on DVE by broadcast tensor_tensor
    (iota==sidx)*ew -- no PSUM-coupled DVE ops anywhere in the stream.
  - h = relu(aggT - t2T*dgw) on 512-col slabs; layer 3 produces node-major
    bf16 h staged via DRAM for pooling.
  - Mean-pool via (1/cnt)-valued one-hot matmuls; 2-layer MLP on-core.
  - Out: per-core [128,4] f32 -> host concat -> [1000,3].
"""

import os
import sys

import numpy as np

sys.path.insert(0, "/opt/trn_rl_repo")

ABL_GATHER = os.environ.get("ABL_GATHER", "0") == "1"   # memset instead of gather
ABL_LSCAT = os.environ.get("ABL_LSCAT", "0") == "1"     # memset instead of onehot
ABL_AG = os.environ.get("ABL_AG", "0") == "1"           # skip AllGather collective
ABL_MM = os.environ.get("ABL_MM", "0") == "1"           # skip per-tile matmuls
ABL_DENSE = os.environ.get("ABL_DENSE", "0") == "1"     # skip dense a/t2 phase
ABL_COMB = os.environ.get("ABL_COMB", "0") == "1"       # skip combine + dgw
ABL_TAIL = os.environ.get("ABL_TAIL", "0") == "1"       # skip pooling + MLP
ABL_FLUSH = os.environ.get("ABL_FLUSH", "0") == "1"     # skip W3b/flush/psC
ABL_GIX = os.environ.get("ABL_GIX", "0") == "1"         # skip gix loads

FULL_CFG = dict(
    n_nodes=100000, n_edges=3200000, n_graphs=1000, hid=64, n_layers=4,
    nc=8, node_pad=12800, ls_t=14,
)


# --------------------------------------------------------------- host prep
def shard_and_pack(inputs, cfg):
    NC, NP = cfg["nc"], cfg["node_pad"]
    NW, NQ = NP // 128, 4
    QROWS = NP * NC // NQ
    G = cfg["n_graphs"]
    GPC = G // NC
    assert QROWS <= 32768

    x = np.asarray(inputs["x"], np.float32)
    ei = np.asarray(inputs["edge_index"], np.int64)
    ew = np.asarray(inputs["edge_attr"], np.float32)
    batch = np.asarray(inputs["batch"], np.int64)
    N = x.shape[0]
    NF = x.shape[1]

    gs = np.searchsorted(batch, np.arange(G + 1))
    nstart = gs[np.arange(NC + 1) * GPC]
    ncnt = np.diff(nstart)
    if ncnt.max() > NP:                                    # rare: grow pad
        NP = int(-(-int(ncnt.max()) // 512) * 512)
        cfg = dict(cfg, node_pad=NP)
        NW = NP // 128
        QROWS = NP * NC // NQ
        assert QROWS <= 32768

    shard_of = np.searchsorted(nstart[1:], np.arange(N), side="right")
    src, dst = ei[0], ei[1]
    e_core = shard_of[dst]
    # quarter of a src node depends only on its shard (QROWS == 2*NP)
    e_q = shard_of[src] * NP // QROWS

    # degree-balanced window packing per core: relabel local node ids so every
    # (window, quarter) edge count is as even as possible.
    newloc = np.zeros(N, np.int64)
    for c in range(NC):
        n_c = int(ncnt[c])
        deg4 = np.zeros((NP, NQ), np.int64)
        selc = e_core == c
        np.add.at(deg4, (dst[selc] - nstart[c], e_q[selc]), 1)
        deg4 = deg4[:n_c]
        order = np.argsort(-deg4.sum(1), kind="stable")
        loads = np.zeros((NW, NQ), np.int64)
        fill = np.zeros(NW, np.int64)
        assign = np.zeros(n_c, np.int64)
        for n in order:
            new_loads = loads + deg4[n]
            over = np.maximum(0, new_loads - 1016).sum(1)
            cand = over * 1e6 + new_loads.max(1).astype(np.float64)
            cand[fill >= 128] = np.inf
            wsel = int(np.argmin(cand))
            assign[n] = wsel * 128 + fill[wsel]
            fill[wsel] += 1
            loads[wsel] += deg4[n]
        newloc[nstart[c]: nstart[c] + n_c] = assign

    spad = shard_of * NP + newloc
    dstloc = newloc[dst]
    e_w = dstloc >> 7

    cnt = np.zeros((NC, NW, NQ), np.int64)
    np.add.at(cnt, (e_core, e_w, e_q), 1)
    T = np.maximum(1, -(-cnt.max(axis=0) // 128))          # [NW, NQ]

    ntiles = int(T.sum())
    LS_T = cfg["ls_t"]
    ntiles_pad = -(-ntiles // LS_T) * LS_T

    per_core = []
    for c in range(NC):
        sel = e_core == c
        s_qi = (spad[src[sel]] % QROWS).astype(np.int64)
        s_q, s_w = e_q[sel], e_w[sel]
        s_off = (dstloc[sel] & 127).astype(np.int64)
        s_ew = ew[sel]

        # window-major slot blocks: tiles ordered (w, q)
        order = np.lexsort((s_off, s_q, s_w))
        s_qi, s_q, s_w, s_off, s_ew = (a[order] for a in (s_qi, s_q, s_w, s_off, s_ew))
        blk_sizes = (T.reshape(-1) * 128)
        blk_base = np.concatenate([[0], np.cumsum(blk_sizes)])[:-1].reshape(NW, NQ)
        key = s_w * NQ + s_q
        grp_start = np.searchsorted(key, np.arange(NW * NQ), side="left")
        slot = blk_base[s_w, s_q] + (np.arange(key.size) - grp_start[key])

        nslots = ntiles * 128
        gidx = np.zeros(nslots, np.int16)
        ewv = np.zeros(nslots, np.float32)
        offv = np.full(nslots, -1, np.int64)
        gidx[slot] = s_qi.astype(np.int16)
        ewv[slot] = s_ew
        offv[slot] = s_off

        gw = np.tile(gidx.reshape(-1, 16).T, (8, 1))       # [128, nslots/16]

        offm = offv.reshape(ntiles, 128).T
        ewm = ewv.reshape(ntiles, 128).T
        sidx = offm.astype(np.int16)          # dst offset in window, -1 = pad
        sidx = np.pad(sidx, ((0, 0), (0, ntiles_pad - ntiles)), constant_values=-1)
        ewm = np.pad(ewm, ((0, 0), (0, ntiles_pad - ntiles)))

        loc = newloc[nstart[c]: nstart[c + 1]]
        xT1 = np.zeros((NF + 1, NP), np.float32)
        xT1[:NF, loc] = x[nstart[c]: nstart[c + 1]].T
        xT1[NF, :] = 1.0

        nb = (batch[nstart[c]: nstart[c + 1]] - c * GPC).astype(np.int64)
        cnts = np.bincount(nb, minlength=GPC).astype(np.float32)
        pool = np.zeros((128, NP), np.float32)
        pool[loc & 127, (loc >> 7) * 128 + nb] = 1.0 / np.maximum(cnts[nb], 1.0)

        per_core.append(dict(gidx=gw, sidx=sidx, ew=ewm, xT1=xT1, pool=pool,
                             ngraphs=GPC))

    meta = dict(T=T, ntiles=ntiles, ntiles_pad=ntiles_pad,
                NW=NW, NQ=NQ, QROWS=QROWS, NF=NF, cfg=cfg)
    return per_core, meta


def weights_map(inputs):
    f32 = np.float32
    vs = np.vstack
    w = {"embWb": vs([np.asarray(inputs["emb_w"], f32),
                      np.asarray(inputs["emb_b"], f32)[None]]),
         "L1b": vs([np.asarray(inputs["lin1_w"], f32),
                    np.asarray(inputs["lin1_b"], f32)[None]]),
         "L2b": vs([np.asarray(inputs["lin2_w"], f32),
                    np.asarray(inputs["lin2_b"], f32)[None]]),
         "ident": np.eye(128, dtype=f32),
         "iotaG": np.tile(np.tile(np.arange(128, dtype=f32),
                          FULL_CFG["ls_t"]), (128, 1))}
    L = np.asarray(inputs["conv_w1"]).shape[0]
    for l in range(L):
        w[f"W1b_{l}"] = vs([np.asarray(inputs["conv_w1"][l], f32),
                            np.asarray(inputs["conv_b1"][l], f32)[None]])
        w[f"W2_{l}"] = np.asarray(inputs["conv_w2"][l], f32)
        w3 = vs([np.asarray(inputs["conv_w3"][l], f32),
                 np.asarray(inputs["conv_b3"][l], f32)[None]])
        w[f"W3b_{l}"] = np.hstack([w3, np.zeros((w3.shape[0], 1), f32)]) \
            if l == 0 else w3
    return w


# --------------------------------------------------------------- builder
def build_graph(meta):
    from concourse import bacc, mybir, tile

    cfg = meta["cfg"]
    NC, H, L = cfg["nc"], cfg["hid"], cfg["n_layers"]
    NP, NW, NQ, QROWS = cfg["node_pad"], meta["NW"], meta["NQ"], meta["QROWS"]
    NF = meta["NF"]
    H2 = 2 * H                                  # padded bf16 a-row (256B)
    T = meta["T"]
    ntiles, ntiles_pad = meta["ntiles"], meta["ntiles_pad"]
    LS_T = cfg["ls_t"]
    TMAXQ = int(T.max())
    TWMAX = int(T.sum(axis=1).max())
    f32, bf16, i16 = mybir.dt.float32, mybir.dt.bfloat16, mybir.dt.int16
    AF = mybir.ActivationFunctionType
    NCH = NP // 512

    nc = bacc.Bacc(num_devices=NC, num_swdge_queues=4)

    gidx_d = nc.declare_dram_parameter("gidx", [128, ntiles * 8], i16, False)
    sidx_d = nc.declare_dram_parameter("sidx", [128, ntiles_pad], bf16, False)
    ew_d = nc.declare_dram_parameter("ew", [128, ntiles_pad], bf16, False)
    xT1_d = nc.declare_dram_parameter("xT1", [NF + 1, NP], bf16, False)
    pool_d = nc.declare_dram_parameter("pool", [128, NP], bf16, False)
    wnames = (["embWb", "L1b", "L2b", "ident", "iotaG"]
              + [f"{p}_{l}" for l in range(L) for p in ("W1b", "W2", "W3b")])
    wshape = {"embWb": [NF + 1, H], "L1b": [H + 1, H], "L2b": [H + 1, 3],
              "ident": [128, 128], "iotaG": [128, LS_T * 128]}
    wdt = {"embWb": bf16, "L1b": f32, "L2b": f32, "ident": f32, "iotaG": bf16}
    for l in range(L):
        wshape[f"W1b_{l}"] = [H + 1, H]
        wshape[f"W2_{l}"] = [H, H]
        wshape[f"W3b_{l}"] = [H + 1, H + 1] if l == 0 else [H + 1, H]
        wdt[f"W1b_{l}"] = wdt[f"W2_{l}"] = wdt[f"W3b_{l}"] = bf16
    wd = {k: nc.declare_dram_parameter(k, wshape[k], wdt[k], False)
          for k in wnames}
    out_d = nc.declare_dram_parameter("out", [128, 4], f32, True)

    with tile.TileContext(nc) as tc:
        with (
            tc.tile_pool(name="res", bufs=1) as res,
            tc.tile_pool(name="dram", bufs=1, space="DRAM") as dram,
            tc.tile_pool(name="stage", bufs=2) as stage,
            tc.tile_pool(name="gbuf", bufs=12) as gpool,
            tc.tile_pool(name="ohbuf", bufs=10) as ohpool,
            tc.tile_pool(name="ixbuf", bufs=3) as ixpool,
            tc.tile_pool(name="scr", bufs=2) as scr,
            tc.tile_pool(name="psA", bufs=2, space="PSUM") as psA,
            tc.tile_pool(name="psB", bufs=2, space="PSUM") as psB,
            tc.tile_pool(name="psC", bufs=3, space="PSUM") as psC,
            tc.tile_pool(name="psG", bufs=1, space="PSUM") as psG,
        ):
            a_loc_pp = [dram.tile([NP, H2], bf16, name=f"a_loc{i}")
                        for i in range(2)]
            a_full_pp = [dram.tile([NP * NC, H2], bf16, name=f"a_full{i}",
                                   addr_space="Shared")
                         for i in range(4)]
            h_nm_d = dram.tile([NW, 128, H], bf16)

            hT = res.tile([H + 1, NP], bf16, tag="hT")
            t2T = res.tile([H, NP], bf16, tag="t2T")
            aggT = res.tile([H + 1, NP], bf16, tag="aggT")
            dgw = res.tile([H, NP], bf16, tag="dgw")
            sidx_s = res.tile([128, ntiles_pad], bf16, tag="sidx")
            ew_s = res.tile([128, ntiles_pad], bf16, tag="ew")
            ones_row = res.tile([1, H], bf16, tag="ones_row")
            wts = {k: res.tile(wshape[k], wdt[k], tag=k, name=k) for k in wnames}
            ident = wts["ident"]

            nc.vector.memset(ones_row[:], 1.0)
            for k in wnames:
                nc.sync.dma_start(wts[k][:], wd[k][:])
            nc.sync.dma_start(sidx_s[:], sidx_d[:])
            nc.sync.dma_start(ew_s[:], ew_d[:])

            # h0T = embWb.T @ xT1 (streamed)
            for ck in range(NCH):
                sl = slice(ck * 512, (ck + 1) * 512)
                xc = stage.tile([NF + 1, 512], bf16, tag="xc")
                nc.sync.dma_start(xc[:], xT1_d[:, sl])
                ps = psA.tile([H, 512], f32, tag="dps")
                nc.tensor.matmul(ps[:], wts["embWb"][:], xc[:],
                                 start=True, stop=True)
                nc.scalar.activation(hT[:H, sl], ps[:], AF.Copy)
            nc.vector.memset(hT[H:H + 1, :], 1.0)

            for l in range(L):
                layer0 = l == 0
                HD = H + 1 if layer0 else H
                a_loc = a_loc_pp[l % 2]
                a_full = a_full_pp[l]

                # ---- dense: a (node-major, ones col at elem 64) -> a_loc; t2T
                for ck in range(NCH if not ABL_DENSE else 0):
                    sl = slice(ck * 512, (ck + 1) * 512)
                    ast = stage.tile([128, 4, H2], bf16, tag="ast")
                    nc.vector.memset(ast[:, :, H:], 0.0)
                    nc.vector.memset(ast[:, :, H:H + 1], 1.0)
                    for j in range(4):
                        wsl = slice(ck * 512 + j * 128, ck * 512 + (j + 1) * 128)
                        pst = psB.tile([128, H], f32, tag="tps")
                        nc.tensor.matmul(pst[:], hT[:, wsl], wts[f"W1b_{l}"][:],
                                         start=True, stop=True)
                        nc.vector.tensor_copy(ast[:, j, :H], pst[:])
                    nc.sync.dma_start(
                        a_loc[sl, :].rearrange("(j p) f -> p j f", p=128), ast[:])
                    ps2 = psA.tile([H, 512], f32, tag="dps")
                    nc.tensor.matmul(ps2[:], wts[f"W2_{l}"][:], hT[:H, sl],
                                     start=True, stop=True)
                    nc.scalar.activation(t2T[:, sl], ps2[:], AF.Copy)

                if not ABL_AG:
                    nc.gpsimd.collective_compute(
                        "AllGather", mybir.AluOpType.bypass,
                        replica_groups=[list(range(NC))],
                        ins=[a_loc[:].opt()], outs=[a_full[:].opt()])

                # ---- window-major gather+scatter stream
                t = 0
                gcall = 0
                ohb = None
                for w in range(NW):
                    wsl = slice(w * 128, (w + 1) * 128)
                    Tw = int(T[w].sum())
                    if not ABL_FLUSH:
                        ps = psC.tile([H + 1, 128], f32, tag="sps")
                        nc.tensor.matmul(ps[:HD, :], wts[f"W3b_{l}"][:],
                                         hT[:, wsl], start=True, stop=False)
                    if not ABL_GIX:
                        gix = ixpool.tile([128, TWMAX * 8], i16, tag="gix")
                        nc.sync.dma_start(gix[:, :Tw * 8],
                                          gidx_d[:, t * 8:(t + Tw) * 8])
                    toff = 0
                    for q in range(NQ):
                        Twq = int(T[w][q])
                        gb = gpool.tile([128, TMAXQ, H2], bf16, tag="gb")
                        if ABL_GATHER:
                            nc.vector.memset(gb[:, :Twq, :], 0.01)
                        else:
                            nc.gpsimd.dma_gather(
                                gb[:, :Twq, :],
                                a_full[q * QROWS:(q + 1) * QROWS, :],
                                gix[:, toff * 8:(toff + Twq) * 8],
                                Twq * 128, Twq * 128, H2,
                                single_packet=False, queue_num=gcall % 4)
                            gcall += 1
                        for i in range(Twq):
                            if t % LS_T == 0:
                                ohb = ohpool.tile([128, LS_T * 128], bf16,
                                                  tag="ohb")
                                g0 = t
                                if ABL_LSCAT:
                                    nc.vector.memset(ohb[:], 0.001)
                                else:
                                    oh3 = ohb[:].rearrange(
                                        "p (t c) -> p t c", c=128)
                                    sib = (sidx_s[:, g0:g0 + LS_T]
                                           .rearrange("p (t o) -> p t o", o=1)
                                           .broadcast_to([128, LS_T, 128]))
                                    ewb = (ew_s[:, g0:g0 + LS_T]
                                           .rearrange("p (t o) -> p t o", o=1)
                                           .broadcast_to([128, LS_T, 128]))
                                    it3 = wts["iotaG"][:].rearrange(
                                        "p (t c) -> p t c", c=128)
                                    nc.vector.tensor_tensor(
                                        oh3, it3, sib, mybir.AluOpType.is_equal)
                                    nc.vector.tensor_tensor(
                                        oh3, oh3, ewb, mybir.AluOpType.mult)
                            oh_sl = ohb[:, (t % LS_T) * 128:(t % LS_T + 1) * 128]
                            last = (q == NQ - 1) and (i == Twq - 1)
                            if not ABL_FLUSH and (not ABL_MM or last):
                                nc.tensor.matmul(
                                    ps[:HD, :], gb[:, i, :HD], oh_sl,
                                    start=False, stop=last)
                            t += 1
                            toff += 1
                    if not ABL_FLUSH:
                        nc.scalar.activation(aggT[:HD, wsl], ps[:HD, :], AF.Copy)

                if layer0 and not ABL_COMB:
                    # dgw = PE-broadcast of deg row (via partition-0 staging)
                    for ck in range(NCH):
                        sl = slice(ck * 512, (ck + 1) * 512)
                        dr = stage.tile([1, 512], bf16, tag="dr")
                        nc.sync.dma_start(dr[:], aggT[H:H + 1, sl])
                        psr = psA.tile([H, 512], f32, tag="dps")
                        nc.tensor.matmul(psr[:], ones_row[:], dr[:],
                                         start=True, stop=True)
                        nc.scalar.activation(dgw[:, sl], psr[:], AF.Copy)

                # ---- combine: h = relu(aggT - t2T*dgw)
                if ABL_COMB:
                    continue
                if l < L - 1:
                    for ck in range(NCH):
                        csl = slice(ck * 512, (ck + 1) * 512)
                        tmp = scr.tile([H, 512], f32, tag="cmb1")
                        nc.vector.tensor_mul(tmp[:], t2T[:, csl], dgw[:, csl])
                        nc.vector.tensor_sub(tmp[:], aggT[:H, csl], tmp[:])
                        nc.scalar.activation(hT[:H, csl], tmp[:], AF.Relu)
                else:
                    for w in range(NW):
                        wsl = slice(w * 128, (w + 1) * 128)
                        tmp = scr.tile([H, 128], f32, tag="cmb2")
                        nc.vector.tensor_mul(tmp[:], t2T[:, wsl], dgw[:, wsl])
                        nc.vector.tensor_sub(tmp[:], aggT[:H, wsl], tmp[:])
                        hTw = scr.tile([H, 128], f32, tag="cmb3")
                        nc.scalar.activation(hTw[:], tmp[:], AF.Relu)
                        pst = psB.tile([128, H], f32, tag="tps")
                        nc.tensor.transpose(pst[:], hTw[:], ident[:H, :H])
                        hst = scr.tile([128, H], bf16, tag="hst")
                        nc.vector.tensor_copy(hst[:], pst[:])
                        nc.sync.dma_start(h_nm_d[w], hst[:])

            # ---- pooling + MLP
            if ABL_TAIL:
                outs0 = stage.tile([128, 4], f32, tag="outs")
                nc.vector.memset(outs0[:], 0.0)
                nc.sync.dma_start(out_d[:], outs0[:])
            else:
                psg = psG.tile([128, H], f32, tag="spsg")
                for w in range(NW):
                    pw = ixpool.tile([128, 128], bf16, tag="pw")
                    nc.sync.dma_start(pw[:], pool_d[:, w * 128:(w + 1) * 128])
                    hb = ixpool.tile([128, H], bf16, tag="hb")
                    nc.sync.dma_start(hb[:], h_nm_d[w])
                    nc.tensor.matmul(psg[:], pw[:], hb[:],
                                     start=(w == 0), stop=(w == NW - 1))
                gx = stage.tile([128, H], f32, tag="gx")
                nc.vector.tensor_copy(gx[:], psg[:])
                pst = psB.tile([128, 128], f32, tag="tps")
                nc.tensor.transpose(pst[:H, :], gx[:], ident[:])
                gxT = stage.tile([H + 1, 128], f32, tag="gxT")
                nc.vector.tensor_copy(gxT[:H, :], pst[:H, :])
                nc.vector.memset(gxT[H:H + 1, :], 1.0)
                ps1 = psB.tile([128, H], f32, tag="tps")
                nc.tensor.matmul(ps1[:], gxT[:], wts["L1b"][:],
                                 start=True, stop=True)
                r1 = stage.tile([128, H], f32, tag="r1")
                nc.scalar.activation(r1[:], ps1[:], AF.Relu)
                pst2 = psB.tile([128, 128], f32, tag="tps")
                nc.tensor.transpose(pst2[:H, :], r1[:], ident[:])
                r1T = stage.tile([H + 1, 128], f32, tag="r1T")
                nc.vector.tensor_copy(r1T[:H, :], pst2[:H, :])
                nc.vector.memset(r1T[H:H + 1, :], 1.0)
                ps2 = psB.tile([128, 4], f32, tag="tps")
                nc.tensor.matmul(ps2[:, :3], r1T[:], wts["L2b"][:],
                                 start=True, stop=True)
                outs = stage.tile([128, 4], f32, tag="outs")
                nc.vector.memset(outs[:], 0.0)
                nc.vector.tensor_copy(outs[:, :3], ps2[:, :3])
                nc.sync.dma_start(out_d[:], outs[:])

    nc.compile()
    return nc


# --------------------------------------------------------------- entry
F32_KEYS = ("L1b", "L2b", "ident")


def make_in_maps(per_core, w, cfg):
    import ml_dtypes
    bf = ml_dtypes.bfloat16
    in_maps = []
    for c in range(cfg["nc"]):
        pc = per_core[c]
        m = {}
        for k, v in w.items():
            m[k] = v if k in F32_KEYS else v.astype(bf)
        m["gidx"], m["sidx"] = pc["gidx"], pc["sidx"].astype(bf)
        m["ew"] = pc["ew"].astype(bf)
        m["xT1"] = pc["xT1"].astype(bf)
        m["pool"] = pc["pool"].astype(bf)
        in_maps.append(m)
    return in_maps


def run(inputs, cfg, trace=False):
    per_core, meta = shard_and_pack(inputs, cfg)
    w = weights_map(inputs)
    in_maps = make_in_maps(per_core, w, cfg)
    nc = build_graph(meta)
    from concourse import bass_utils
    res = bass_utils.run_bass_kernel_spmd(
        nc, in_maps, core_ids=list(range(cfg["nc"])), trace=trace)
    outs = [np.asarray(res.results[c]["out"])[:per_core[c]["ngraphs"], :3]
            for c in range(cfg["nc"])]
    return np.concatenate(outs, 0).astype(np.float32), res


def kernel(**inputs):
    out, _ = run(inputs, FULL_CFG)
    return out
